# revision 27
# baseline (speedup 1.0000x reference)
"""Trainium2 Bass kernel for CustomStrainEnergyLoss.

Math (d = y_pred - y_true, f = clipped fracture_idx):
    pred_int_b - true_int_b = masked_trapz(d)                 (linearity)
    t_b  = sum_j 0.5*dx_j*(d_{b,j} + d_{b,j+1}) * [j < f_b]
    out  = mean_b(t_b^2)

For the uniform grid (x_values = arange, always true for this problem), with
m1 = [i<f], m2 = [i<=f] and m2 - m1 = [i==f]:
    sum_j (d_j + d_{j+1})*m1_j = sum_i d_i*m1_i + sum_i d_i*m2_i - d_0
                               = 2*sum_i d_i*[i<f] + (d_f - d_0)
so the device does just TWO DVE passes per element (memory-bound at the
~358 GB/s per-core HBM limit):
    d = yp - yt                          (tensor_sub)
    A = sum((iota < f) * d)              (scalar_tensor_tensor, fused accum)
then S = 2A + hcol per row ([128,1] ops), out = S^2.  hcol = d_f - d_0 is an
O(B) host-side gather. The 0.5*dx weight folds into a host-side scalar on the
final mean. A general path (non-uniform dx) multiplies the trapezoid segments
by a replicated 0.5*dx row on device instead.

Sharding: pure data parallel, 512 rows per core across 8 cores, y_pred on the
sync-engine HWDGE ring and y_true on the scalar-engine ring (alternating per
chunk); per-core row results [128, 4] are squared on device, summed on host.

This neuronx-cc build rejects instructions with >1 sync wait, so
_split_excess_waits moves extra waits onto same-engine NoOps post-schedule.
"""

import numpy as np

from concourse import bass
import concourse.mybir as mybir
from concourse.tile import TileContext
from concourse.bass_utils import run_bass_kernel_spmd

B, N = 4096, 8192
NCORES = 8
BS = B // NCORES          # 512 rows per core
P = 128                   # partitions
RT = BS // P              # 4 row-tiles per core
K = 2048                  # column chunk
NCH = N // K              # 4 chunks

_nc_cache = {}


def _split_excess_waits(nc, maxw: int = 1):
    """Workaround for this neuronx-cc build: walrus codegen rejects any
    instruction carrying more than one sync wait ("Too many sync wait
    commands" in setupSyncWait). Move extra waits onto same-engine NoOps
    inserted immediately before the instruction (sequencer executes them in
    order, so semantics are unchanged)."""
    for b in nc.main_func.blocks:
        newlist = []
        for ins in b.instructions:
            si = ins.sync_info
            ow = list(si.on_wait) if si else []
            if len(ow) > maxw:
                extra, keep = ow[:len(ow) - maxw], ow[len(ow) - maxw:]
                for i in range(0, len(extra), maxw):
                    nop = mybir.InstNoOp(
                        name=nc.get_next_instruction_name(), ins=[], outs=[])
                    nop.engine = ins.engine
                    nop.sync_info = mybir.SyncInfo(
                        on_wait=list(extra[i:i + maxw]), on_update=[])
                    nc.register_instruction(nop)
                    newlist.append(nop)
                ins.sync_info = mybir.SyncInfo(
                    on_wait=list(keep), on_update=list(si.on_update))
            newlist.append(ins)
        b.instructions[:] = newlist
    return nc


def build_nc_v2(reps: int = 1, io_bufs: int = 3, cmp_bufs: int = 2):
    """Uniform-dx fast path.

    S_b = sum_i d_i*[i<f_b] + sum_i d_i*[i<=f_b] - d_0   (all over full rows)
    Per [128, 4096] chunk: one tensor_sub + two fused STT mask-reduces.
    2 MiB DMA loads, y_pred on the sync HWDGE ring, y_true on the scalar ring.
    """
    f32 = mybir.dt.float32
    K2 = 4096
    NCH2 = N // K2  # 2
    nc = bass.Bass()
    yp = nc.declare_dram_parameter("yp", [BS, N], f32, isOutput=False)
    yt = nc.declare_dram_parameter("yt", [BS, N], f32, isOutput=False)
    fcl = nc.declare_dram_parameter("fcl", [BS, 1], f32, isOutput=False)
    o_sq = nc.declare_dram_parameter("o_sq", [P, RT], f32, isOutput=True)

    with TileContext(nc) as tc:
        with tc.tile_pool(name="pio", bufs=io_bufs) as pio, \
             tc.tile_pool(name="pcmp", bufs=cmp_bufs) as pc, \
             tc.tile_pool(name="pq", bufs=1) as pq, \
             tc.tile_pool(name="pers", bufs=1) as pp:
            iotas = []
            for c in range(NCH2):
                it = pp.tile([P, K2], f32, tag=f"iota{c}")
                nc.gpsimd.iota(
                    it, pattern=[[1, K2]], base=c * K2, channel_multiplier=0,
                    allow_small_or_imprecise_dtypes=True,
                )
                iotas.append(it)
            outt = pp.tile([P, RT], f32, tag="outt")

            for _rep in range(reps):
                for rt in range(RT):
                    r0 = rt * P
                    fcol = pc.tile([P, 1], f32, tag="fcol")
                    nc.sync.dma_start(out=fcol, in_=fcl[r0:r0 + P, :])
                    pab = pc.tile([P, 2 * NCH2], f32, tag="pab")
                    d0 = pc.tile([P, 1], f32, tag="d0")
                    for c in range(NCH2):
                        c0 = c * K2
                        ypt = pio.tile([P, K2], f32, tag="ypt")
                        ytt = pio.tile([P, K2], f32, tag="ytt")
                        nc.sync.dma_start(out=ypt, in_=yp[r0:r0 + P, c0:c0 + K2])
                        nc.scalar.dma_start(out=ytt, in_=yt[r0:r0 + P, c0:c0 + K2])
                        d = pc.tile([P, K2], f32, tag="d")
                        nc.vector.tensor_sub(out=d, in0=ypt, in1=ytt)
                        if c == 0:
                            nc.vector.tensor_copy(out=d0, in_=d[:, 0:1])
                        q = pq.tile([P, K2], f32, tag="q")
                        nc.vector.scalar_tensor_tensor(
                            out=q, in0=iotas[c], scalar=fcol, in1=d,
                            op0=mybir.AluOpType.is_lt, op1=mybir.AluOpType.mult,
                            accum_out=pab[:, c:c + 1],
                        )
                        nc.vector.scalar_tensor_tensor(
                            out=q, in0=iotas[c], scalar=fcol, in1=d,
                            op0=mybir.AluOpType.is_le, op1=mybir.AluOpType.mult,
                            accum_out=pab[:, NCH2 + c:NCH2 + c + 1],
                        )
                    ssum = pc.tile([P, 1], f32, tag="ssum")
                    nc.vector.tensor_reduce(
                        out=ssum, in_=pab, axis=mybir.AxisListType.X, op=mybir.AluOpType.add
                    )
                    st = pc.tile([P, 1], f32, tag="st")
                    nc.vector.tensor_sub(out=st, in0=ssum, in1=d0)
                    nc.vector.tensor_mul(out=outt[:, rt:rt + 1], in0=st, in1=st)
            nc.sync.dma_start(out=o_sq[:, :], in_=outt[:, :])
    return _split_excess_waits(nc)


def build_nc_v3(reps: int = 1, io_bufs: int = 3, cmp_bufs: int = 2,
                chunk_k: int = 4096, d_bufs: int = 2, batched_fh: bool = True,
                alt_rings: bool = False):
    """Uniform-dx fast path, 2 DVE passes per element.

    Identity: with m1 = [i<f], m2 = [i<=f],  m2 - m1 = [i==f], so
        S_b = sum_i d_i*m1 + sum_i d_i*m2 - d_0 = 2*sum_i d_i*[i<f] + (d_f - d_0).
    The host supplies hcol = d_f - d_0 per row (an O(B) gather); the device
    does d = yp - yt and ONE fused mask-reduce per chunk, then
    S = 2*A + hcol, out = S^2.
    """
    f32 = mybir.dt.float32
    K2 = chunk_k
    NCH2 = N // K2
    nc = bass.Bass()
    yp = nc.declare_dram_parameter("yp", [BS, N], f32, isOutput=False)
    yt = nc.declare_dram_parameter("yt", [BS, N], f32, isOutput=False)
    fcl = nc.declare_dram_parameter("fcl", [BS, 1], f32, isOutput=False)
    hcl = nc.declare_dram_parameter("hcl", [BS, 1], f32, isOutput=False)
    o_sq = nc.declare_dram_parameter("o_sq", [P, RT], f32, isOutput=True)
    # [512,1] viewed as [128, RT]: column rt holds rows rt*128..rt*128+127
    fview = fcl.rearrange("(rt p) one -> p (rt one)", p=P)
    hview = hcl.rearrange("(rt p) one -> p (rt one)", p=P)

    with TileContext(nc) as tc:
        with tc.tile_pool(name="pio", bufs=io_bufs) as pio, \
             tc.tile_pool(name="pcmp", bufs=cmp_bufs) as pc, \
             tc.tile_pool(name="pd", bufs=d_bufs) as pd, \
             tc.tile_pool(name="pq", bufs=1) as pq, \
             tc.tile_pool(name="pers", bufs=1) as pp:
            iotas = []
            for c in range(NCH2):
                it = pp.tile([P, K2], f32, tag=f"iota{c}")
                nc.gpsimd.iota(
                    it, pattern=[[1, K2]], base=c * K2, channel_multiplier=0,
                    allow_small_or_imprecise_dtypes=True,
                )
                iotas.append(it)
            outt = pp.tile([P, RT], f32, tag="outt")

            for _rep in range(reps):
                if batched_fh:
                    fcol4 = pc.tile([P, RT], f32, tag="fcol4")
                    nc.sync.dma_start(out=fcol4, in_=fview)
                    hcol4 = pc.tile([P, RT], f32, tag="hcol4")
                    nc.sync.dma_start(out=hcol4, in_=hview)
                for rt in range(RT):
                    r0 = rt * P
                    if not batched_fh:
                        fcol4 = pc.tile([P, RT], f32, tag="fcol4")
                        nc.sync.dma_start(out=fcol4[:, rt:rt + 1], in_=fcl[r0:r0 + P, :])
                        hcol4 = pc.tile([P, RT], f32, tag="hcol4")
                        nc.sync.dma_start(out=hcol4[:, rt:rt + 1], in_=hcl[r0:r0 + P, :])
                    pab = pc.tile([P, NCH2], f32, tag="pab")
                    for c in range(NCH2):
                        c0 = c * K2
                        ypt = pio.tile([P, K2], f32, tag="ypt")
                        ytt = pio.tile([P, K2], f32, tag="ytt")
                        e0, e1 = (nc.sync, nc.scalar)
                        if alt_rings and (rt * NCH2 + c) % 2 == 1:
                            e0, e1 = (nc.scalar, nc.sync)
                        e0.dma_start(out=ypt, in_=yp[r0:r0 + P, c0:c0 + K2])
                        e1.dma_start(out=ytt, in_=yt[r0:r0 + P, c0:c0 + K2])
                        d = pd.tile([P, K2], f32, tag="d")
                        nc.vector.tensor_sub(out=d, in0=ypt, in1=ytt)
                        q = pq.tile([P, K2], f32, tag="q")
                        nc.vector.scalar_tensor_tensor(
                            out=q, in0=iotas[c], scalar=fcol4[:, rt:rt + 1], in1=d,
                            op0=mybir.AluOpType.is_lt, op1=mybir.AluOpType.mult,
                            accum_out=pab[:, c:c + 1],
                        )
                    ssum = pc.tile([P, 1], f32, tag="ssum")
                    if NCH2 > 1:
                        nc.vector.tensor_reduce(
                            out=ssum, in_=pab, axis=mybir.AxisListType.X,
                            op=mybir.AluOpType.add,
                        )
                    else:
                        ssum = pab
                    st = pc.tile([P, 1], f32, tag="st")
                    nc.vector.scalar_tensor_tensor(
                        out=st, in0=ssum, scalar=2.0, in1=hcol4[:, rt:rt + 1],
                        op0=mybir.AluOpType.mult, op1=mybir.AluOpType.add,
                    )
                    nc.vector.tensor_mul(out=outt[:, rt:rt + 1], in0=st, in1=st)
            nc.sync.dma_start(out=o_sq[:, :], in_=outt[:, :])
    return _split_excess_waits(nc)


def build_nc_v4(bounds, chunk_k: int = 512, reps: int = 1, io_bufs: int = 4,
                cmp_bufs: int = 2, d_bufs: int = 2, full_upto=None,
                do_dma: bool = True, do_compute: bool = True):
    """Ragged fast path: rows are pre-sorted by fracture_idx on host, so for
    column chunk c only the partition-suffix of rows with f > c*K contributes;
    everything below the suffix is masked to exactly 0 anyway. We skip both
    the DMA and the vector work for those rows: traffic drops from B*N to
    ~sum(f) elements (~2x for uniform f). `bounds[rt][c]` is the first
    partition that needs chunk c in row-tile rt (shared across cores = min
    over cores, so one SPMD NEFF serves all 8).
    Per-tile accumulator pab is memset to 0, so skipped (row, chunk) cells
    contribute nothing; loaded-but-masked rows contribute (iota<f)*d = 0.
    """
    f32 = mybir.dt.float32
    K2 = chunk_k
    NCH2 = N // K2
    nc = bass.Bass()
    yp = nc.declare_dram_parameter("yp", [BS, N], f32, isOutput=False)
    yt = nc.declare_dram_parameter("yt", [BS, N], f32, isOutput=False)
    fcl = nc.declare_dram_parameter("fcl", [BS, 1], f32, isOutput=False)
    hcl = nc.declare_dram_parameter("hcl", [BS, 1], f32, isOutput=False)
    o_sq = nc.declare_dram_parameter("o_sq", [P, RT], f32, isOutput=True)
    fview = fcl.rearrange("(rt p) one -> p (rt one)", p=P)
    hview = hcl.rearrange("(rt p) one -> p (rt one)", p=P)

    with TileContext(nc) as tc:
        with tc.tile_pool(name="pio", bufs=io_bufs) as pio, \
             tc.tile_pool(name="pcmp", bufs=cmp_bufs) as pc, \
             tc.tile_pool(name="pd", bufs=d_bufs) as pd, \
             tc.tile_pool(name="pq", bufs=1) as pq, \
             tc.tile_pool(name="pers", bufs=1) as pp:
            iota = pp.tile([P, N], f32, tag="iota")
            nc.gpsimd.iota(
                iota, pattern=[[1, N]], base=0, channel_multiplier=0,
                allow_small_or_imprecise_dtypes=True,
            )
            outt = pp.tile([P, RT], f32, tag="outt")

            # Compute APs must start on a 32-partition quadrant boundary, so
            # vector ops below run on [ps32:P) while DMA fills only [ps:P).
            # Rows in [ps32, ps) are masked to 0 by (iota < f); memset every
            # io buffer once so their stale contents are finite (0*x == 0).
            for _i in range(io_bufs):
                zyp = pio.tile([P, K2], f32, tag="ypt")
                nc.vector.memset(zyp, 0.0)
                zyt = pio.tile([P, K2], f32, tag="ytt")
                nc.vector.memset(zyt, 0.0)

            flip = 0
            for _rep in range(reps):
                fcol4 = pc.tile([P, RT], f32, tag="fcol4")
                nc.sync.dma_start(out=fcol4, in_=fview)
                hcol4 = pc.tile([P, RT], f32, tag="hcol4")
                nc.sync.dma_start(out=hcol4, in_=hview)
                for rt in range(RT):
                    r0 = rt * P
                    pab = pc.tile([P, NCH2], f32, tag="pab")
                    nc.vector.memset(pab, 0.0)
                    for c in range(NCH2):
                        ps = bounds[rt][c]
                        if ps >= P:
                            break  # f sorted -> later chunks needed by nobody
                        c0 = c * K2
                        ypt = pio.tile([P, K2], f32, tag="ypt")
                        ytt = pio.tile([P, K2], f32, tag="ytt")
                        e0, e1 = (nc.sync, nc.scalar) if flip == 0 else (nc.scalar, nc.sync)
                        flip ^= 1
                        if do_dma:
                            e0.dma_start(out=ypt[ps:P, :], in_=yp[r0 + ps:r0 + P, c0:c0 + K2])
                            e1.dma_start(out=ytt[ps:P, :], in_=yt[r0 + ps:r0 + P, c0:c0 + K2])
                        if not do_compute:
                            continue
                        # Compute on all 128 partitions (nonzero partition
                        # starts are limited to one 32-quadrant); rows below
                        # ps hold stale-but-finite data and mask to 0.
                        d = pd.tile([P, K2], f32, tag="d")
                        if full_upto is not None and c < full_upto[rt]:
                            # Chunk lies inside every row's prefix: fuse
                            # subtract + row-sum in one DVE pass, no mask.
                            nc.vector.tensor_tensor_reduce(
                                out=d, in0=ypt, in1=ytt, scale=1.0, scalar=0.0,
                                op0=mybir.AluOpType.subtract,
                                op1=mybir.AluOpType.add,
                                accum_out=pab[:, c:c + 1],
                            )
                            continue
                        nc.vector.tensor_sub(out=d, in0=ypt, in1=ytt)
                        q = pq.tile([P, K2], f32, tag="q")
                        nc.vector.scalar_tensor_tensor(
                            out=q, in0=iota[:, c0:c0 + K2],
                            scalar=fcol4[:, rt:rt + 1], in1=d,
                            op0=mybir.AluOpType.is_lt, op1=mybir.AluOpType.mult,
                            accum_out=pab[:, c:c + 1],
                        )
                    ssum = pc.tile([P, 1], f32, tag="ssum")
                    nc.vector.tensor_reduce(
                        out=ssum, in_=pab, axis=mybir.AxisListType.X,
                        op=mybir.AluOpType.add,
                    )
                    st = pc.tile([P, 1], f32, tag="st")
                    nc.vector.scalar_tensor_tensor(
                        out=st, in0=ssum, scalar=2.0, in1=hcol4[:, rt:rt + 1],
                        op0=mybir.AluOpType.mult, op1=mybir.AluOpType.add,
                    )
                    nc.vector.tensor_mul(out=outt[:, rt:rt + 1], in0=st, in1=st)
            nc.sync.dma_start(out=o_sq[:, :], in_=outt[:, :])
    return _split_excess_waits(nc)


def build_nc_v5(widths, full_upto, chunk_k: int = 2048, reps: int = 1,
                io_bufs: int = 2, cmp_bufs: int = 2, d_bufs: int = 1,
                group_rows: int = 8, do_dma: bool = True,
                do_compute: bool = True):
    """Ragged fast path, row-group full-prefix DMA.

    v4's [row-suffix x 512-col] DMAs made 2 KiB descriptors and died on
    per-descriptor overhead (250 GB/s dma-only). Here each DMA loads
    [group_rows x W_g] where W_g = the group's shared prefix width, so every
    descriptor is one FULL row prefix (up to 32 KiB). widths[rt][g] (elems,
    nondecreasing in g) and full_upto[rt] (chunks fully inside every row's
    prefix) are host-computed from sorted f, shared across cores (max over
    cores) so one SPMD NEFF serves all 8.
    Full chunks need no mask: row-sums of yp and yt accumulate separately
    (single-input tensor_reduce; tensor_tensor_reduce trips "ISA wrong
    length" in this neuronx-cc build) and subtract at the end. Boundary
    chunks use sub + masked STT. Rows keep real data on [0, W_g); columns
    beyond W_g hold stale-but-finite values (prologue memset) that the
    (iota < f) mask zeroes.
    """
    f32 = mybir.dt.float32
    K2 = chunk_k
    GR = group_rows
    NG = P // GR
    nc = bass.Bass()
    yp = nc.declare_dram_parameter("yp", [BS, N], f32, isOutput=False)
    yt = nc.declare_dram_parameter("yt", [BS, N], f32, isOutput=False)
    fcl = nc.declare_dram_parameter("fcl", [BS, 1], f32, isOutput=False)
    hcl = nc.declare_dram_parameter("hcl", [BS, 1], f32, isOutput=False)
    o_sq = nc.declare_dram_parameter("o_sq", [P, RT], f32, isOutput=True)
    fview = fcl.rearrange("(rt p) one -> p (rt one)", p=P)
    hview = hcl.rearrange("(rt p) one -> p (rt one)", p=P)

    wmax = [max(widths[rt]) for rt in range(RT)]

    with TileContext(nc) as tc:
        with tc.tile_pool(name="pio", bufs=io_bufs) as pio, \
             tc.tile_pool(name="pcmp", bufs=cmp_bufs) as pc, \
             tc.tile_pool(name="pd", bufs=d_bufs) as pd, \
             tc.tile_pool(name="pq", bufs=1) as pq, \
             tc.tile_pool(name="pers", bufs=1) as pp:
            iota = pp.tile([P, N], f32, tag="iota")
            nc.gpsimd.iota(
                iota, pattern=[[1, N]], base=0, channel_multiplier=0,
                allow_small_or_imprecise_dtypes=True,
            )
            outt = pp.tile([P, RT], f32, tag="outt")
            nc.vector.memset(outt, 0.0)

            for _i in range(io_bufs):
                zyp = pio.tile([P, N], f32, tag="ypt")
                nc.vector.memset(zyp, 0.0)
                zyt = pio.tile([P, N], f32, tag="ytt")
                nc.vector.memset(zyt, 0.0)

            flip = 0
            for _rep in range(reps):
                fcol4 = pc.tile([P, RT], f32, tag="fcol4")
                nc.sync.dma_start(out=fcol4, in_=fview)
                hcol4 = pc.tile([P, RT], f32, tag="hcol4")
                nc.sync.dma_start(out=hcol4, in_=hview)
                for rt in range(RT):
                    r0 = rt * P
                    wt = wmax[rt]
                    nch_t = (wt + K2 - 1) // K2
                    ypt = pio.tile([P, N], f32, tag="ypt")
                    ytt = pio.tile([P, N], f32, tag="ytt")
                    if do_dma:
                        for g in range(NG):
                            wg = widths[rt][g]
                            if wg <= 0:
                                continue
                            p0 = g * GR
                            e0, e1 = (nc.sync, nc.scalar) if flip == 0 else (nc.scalar, nc.sync)
                            flip ^= 1
                            e0.dma_start(out=ypt[p0:p0 + GR, :wg],
                                         in_=yp[r0 + p0:r0 + p0 + GR, :wg])
                            e1.dma_start(out=ytt[p0:p0 + GR, :wg],
                                         in_=yt[r0 + p0:r0 + p0 + GR, :wg])
                    if not do_compute:
                        continue
                    # pabp accumulates +sum(yp)/-masked contributions per
                    # chunk column; pabt the matching yt sums (full chunks
                    # only; boundary chunks fold the whole (yp-yt) masked
                    # sum into pabp and leave pabt's column at 0).
                    pabp = pc.tile([P, N // K2], f32, tag="pabp")
                    nc.vector.memset(pabp, 0.0)
                    pabt = pc.tile([P, N // K2], f32, tag="pabt")
                    nc.vector.memset(pabt, 0.0)
                    for c in range(nch_t):
                        c0 = c * K2
                        w = min(K2, wt - c0)
                        if c < full_upto[rt]:
                            nc.vector.tensor_reduce(
                                out=pabp[:, c:c + 1], in_=ypt[:, c0:c0 + w],
                                axis=mybir.AxisListType.X, op=mybir.AluOpType.add,
                            )
                            nc.vector.tensor_reduce(
                                out=pabt[:, c:c + 1], in_=ytt[:, c0:c0 + w],
                                axis=mybir.AxisListType.X, op=mybir.AluOpType.add,
                            )
                            continue
                        d = pd.tile([P, K2], f32, tag="d")
                        nc.vector.tensor_sub(out=d[:, :w], in0=ypt[:, c0:c0 + w],
                                             in1=ytt[:, c0:c0 + w])
                        q = pq.tile([P, K2], f32, tag="q")
                        nc.vector.scalar_tensor_tensor(
                            out=q[:, :w], in0=iota[:, c0:c0 + w],
                            scalar=fcol4[:, rt:rt + 1], in1=d[:, :w],
                            op0=mybir.AluOpType.is_lt, op1=mybir.AluOpType.mult,
                            accum_out=pabp[:, c:c + 1],
                        )
                    ssp = pc.tile([P, 1], f32, tag="ssp")
                    nc.vector.tensor_reduce(
                        out=ssp, in_=pabp, axis=mybir.AxisListType.X,
                        op=mybir.AluOpType.add,
                    )
                    sst = pc.tile([P, 1], f32, tag="sst")
                    nc.vector.tensor_reduce(
                        out=sst, in_=pabt, axis=mybir.AxisListType.X,
                        op=mybir.AluOpType.add,
                    )
                    sdf = pc.tile([P, 1], f32, tag="sdf")
                    nc.vector.tensor_sub(out=sdf, in0=ssp, in1=sst)
                    st = pc.tile([P, 1], f32, tag="st")
                    nc.vector.scalar_tensor_tensor(
                        out=st, in0=sdf, scalar=2.0, in1=hcol4[:, rt:rt + 1],
                        op0=mybir.AluOpType.mult, op1=mybir.AluOpType.add,
                    )
                    nc.vector.tensor_mul(out=outt[:, rt:rt + 1], in0=st, in1=st)
            nc.sync.dma_start(out=o_sq[:, :], in_=outt[:, :])
    return _split_excess_waits(nc)


def make_in_maps_v5(y_pred, y_true, x_values, fracture_idx, chunk_k: int = 2048,
                    group_rows: int = 8, align: int = 128):
    """Sharding + group widths for v5. Returns None when dx non-uniform."""
    x = np.asarray(x_values, dtype=np.float32)
    dx = np.diff(x)
    if not (dx.size > 0 and bool(np.all(dx == dx[0]))):
        return None
    scale = float(0.5 * dx[0]) ** 2 / B

    y_pred = np.ascontiguousarray(np.asarray(y_pred, dtype=np.float32))
    y_true = np.ascontiguousarray(np.asarray(y_true, dtype=np.float32))
    idx = np.clip(np.asarray(fracture_idx).astype(np.int64), 0, N - 1)
    rows_all = np.arange(B)
    d_f = y_pred[rows_all, idx] - y_true[rows_all, idx]
    d_0 = y_pred[:, 0] - y_true[:, 0]
    h = (d_f - d_0).astype(np.float32)
    f = idx.astype(np.float32)

    order = np.argsort(idx, kind="stable")
    in_maps = []
    fmat = np.empty((NCORES, BS), np.int64)
    for m in range(NCORES):
        rows = order[m::NCORES]
        fmat[m] = idx[rows]
        in_maps.append({
            "yp": np.ascontiguousarray(y_pred[rows]),
            "yt": np.ascontiguousarray(y_true[rows]),
            "fcl": np.ascontiguousarray(f[rows].reshape(BS, 1)),
            "hcl": np.ascontiguousarray(h[rows].reshape(BS, 1)),
        })
    GR = group_rows
    NG = P // GR
    widths = []
    full_upto = []
    for rt in range(RT):
        row_w = []
        for g in range(NG):
            lo = rt * P + g * GR
            w = int(fmat[:, lo:lo + GR].max())  # max f over group, all cores
            w = min(N, -(-w // align) * align)
            row_w.append(w)
        widths.append(tuple(row_w))
        minf = int(fmat[:, rt * P].min())
        full_upto.append(minf // chunk_k)
    return in_maps, tuple(widths), tuple(full_upto), scale


def build_nc_v6(widths, full_upto, chunk_k: int = 2048, reps: int = 1,
                io_bufs: int = 2, cmp_bufs: int = 2, d_bufs: int = 1,
                group_rows: int = 16, nqueues: int = 2,
                do_dma: bool = True, do_compute: bool = True,
                act_full: bool = False, gp_sub: bool = False,
                fh_gp: bool = False, fuse_io: bool = False):
    """Ragged fast path, engine-spread row-group DMA.

    Per-dma_start throughput is (partition span / 8) x ~27 GB/s SDMA engines
    (measured: 8-row groups 80 GB/s, 32-row 232 GB/s, 128-row ~360 GB/s), so
    v6 strides each group's GR rows across partitions g, g+NG, g+2*NG, ...
    (NG = 128/GR): every group DMA then touches all 16 SDMA engines at full
    rate while keeping the fine per-group prefix widths (~2% overshoot).
    DRAM stays row-sorted and contiguous; fcl/hcl are host-permuted to
    partition order, so compute is identical to v5.
    """
    f32 = mybir.dt.float32
    K2 = chunk_k
    GR = group_rows
    NG = P // GR
    nc = bass.Bass()
    if fuse_io:
        yy = nc.declare_dram_parameter("yy", [BS, 2 * N], f32, isOutput=False)
        yy3 = yy.rearrange("b (t n) -> b t n", t=2)
    else:
        yp = nc.declare_dram_parameter("yp", [BS, N], f32, isOutput=False)
        yt = nc.declare_dram_parameter("yt", [BS, N], f32, isOutput=False)
    fcl = nc.declare_dram_parameter("fcl", [BS, 1], f32, isOutput=False)
    hcl = nc.declare_dram_parameter("hcl", [BS, 1], f32, isOutput=False)
    o_sq = nc.declare_dram_parameter("o_sq", [P, RT], f32, isOutput=True)
    fview = fcl.rearrange("(rt p) one -> p (rt one)", p=P)
    hview = hcl.rearrange("(rt p) one -> p (rt one)", p=P)

    wmax = [max(widths[rt]) for rt in range(RT)]
    queues = [nc.sync, nc.scalar, nc.gpsimd][:nqueues]

    with TileContext(nc) as tc:
        with tc.tile_pool(name="pio", bufs=io_bufs) as pio, \
             tc.tile_pool(name="pcmp", bufs=cmp_bufs) as pc, \
             tc.tile_pool(name="pd", bufs=d_bufs) as pd, \
             tc.tile_pool(name="pq", bufs=1) as pq, \
             tc.tile_pool(name="pers", bufs=1) as pp:
            iota = pp.tile([P, N], f32, tag="iota")
            nc.gpsimd.iota(
                iota, pattern=[[1, N]], base=0, channel_multiplier=0,
                allow_small_or_imprecise_dtypes=True,
            )
            outt = pp.tile([P, RT], f32, tag="outt")
            nc.vector.memset(outt, 0.0)

            if fuse_io:
                for _i in range(io_bufs):
                    zyy = pio.tile([P, 2 * N], f32, tag="yyt")
                    nc.vector.memset(zyy, 0.0)
            else:
                for _i in range(io_bufs):
                    zyp = pio.tile([P, N], f32, tag="ypt")
                    nc.vector.memset(zyp, 0.0)
                    zyt = pio.tile([P, N], f32, tag="ytt")
                    nc.vector.memset(zyt, 0.0)

            qi = 0
            for _rep in range(reps):
                fq = nc.gpsimd if fh_gp else nc.sync
                fcol4 = pc.tile([P, RT], f32, tag="fcol4")
                fq.dma_start(out=fcol4, in_=fview)
                hcol4 = pc.tile([P, RT], f32, tag="hcol4")
                fq.dma_start(out=hcol4, in_=hview)
                for rt in range(RT):
                    r0 = rt * P
                    wt = wmax[rt]
                    nch_t = (wt + K2 - 1) // K2
                    if fuse_io:
                        yyt = pio.tile([P, 2 * N], f32, tag="yyt")
                        yyt3 = yyt.rearrange("p (t n) -> p t n", t=2)
                        ypt = yyt[:, 0:N]
                        ytt = yyt[:, N:2 * N]
                        if do_dma:
                            for g in range(NG):
                                wg = widths[rt][g]
                                if wg <= 0:
                                    continue
                                s0 = r0 + GR * g
                                queues[qi % nqueues].dma_start(
                                    out=yyt3[g::NG, :, :wg],
                                    in_=yy3[s0:s0 + GR, :, :wg])
                                qi += 1
                    else:
                        ypt = pio.tile([P, N], f32, tag="ypt")
                        ytt = pio.tile([P, N], f32, tag="ytt")
                        if do_dma:
                            for g in range(NG):
                                wg = widths[rt][g]
                                if wg <= 0:
                                    continue
                                s0 = r0 + GR * g
                                queues[qi % nqueues].dma_start(
                                    out=ypt[g::NG, :wg], in_=yp[s0:s0 + GR, :wg])
                                queues[(qi + 1) % nqueues].dma_start(
                                    out=ytt[g::NG, :wg], in_=yt[s0:s0 + GR, :wg])
                                qi += 2
                    if not do_compute:
                        continue
                    pabp = pc.tile([P, N // K2], f32, tag="pabp")
                    nc.vector.memset(pabp, 0.0)
                    pabt = pc.tile([P, N // K2], f32, tag="pabt")
                    nc.vector.memset(pabt, 0.0)
                    for c in range(nch_t):
                        c0 = c * K2
                        w = min(K2, wt - c0)
                        if c < full_upto[rt]:
                            if act_full:
                                ascr = pq.tile([P, K2], f32, tag="ascr")
                                nc.scalar.activation(
                                    out=ascr[:, :w], in_=ypt[:, c0:c0 + w],
                                    func=mybir.ActivationFunctionType.Copy,
                                    accum_out=pabp[:, c:c + 1],
                                )
                                nc.scalar.activation(
                                    out=ascr[:, :w], in_=ytt[:, c0:c0 + w],
                                    func=mybir.ActivationFunctionType.Copy,
                                    accum_out=pabt[:, c:c + 1],
                                )
                                continue
                            nc.vector.tensor_reduce(
                                out=pabp[:, c:c + 1], in_=ypt[:, c0:c0 + w],
                                axis=mybir.AxisListType.X, op=mybir.AluOpType.add,
                            )
                            nc.vector.tensor_reduce(
                                out=pabt[:, c:c + 1], in_=ytt[:, c0:c0 + w],
                                axis=mybir.AxisListType.X, op=mybir.AluOpType.add,
                            )
                            continue
                        d = pd.tile([P, K2], f32, tag="d")
                        sub_eng = nc.gpsimd if gp_sub else nc.vector
                        sub_eng.tensor_sub(out=d[:, :w], in0=ypt[:, c0:c0 + w],
                                           in1=ytt[:, c0:c0 + w])
                        q = pq.tile([P, K2], f32, tag="q")
                        nc.vector.scalar_tensor_tensor(
                            out=q[:, :w], in0=iota[:, c0:c0 + w],
                            scalar=fcol4[:, rt:rt + 1], in1=d[:, :w],
                            op0=mybir.AluOpType.is_lt, op1=mybir.AluOpType.mult,
                            accum_out=pabp[:, c:c + 1],
                        )
                    ssp = pc.tile([P, 1], f32, tag="ssp")
                    nc.vector.tensor_reduce(
                        out=ssp, in_=pabp, axis=mybir.AxisListType.X,
                        op=mybir.AluOpType.add,
                    )
                    sst = pc.tile([P, 1], f32, tag="sst")
                    nc.vector.tensor_reduce(
                        out=sst, in_=pabt, axis=mybir.AxisListType.X,
                        op=mybir.AluOpType.add,
                    )
                    sdf = pc.tile([P, 1], f32, tag="sdf")
                    nc.vector.tensor_sub(out=sdf, in0=ssp, in1=sst)
                    st = pc.tile([P, 1], f32, tag="st")
                    nc.vector.scalar_tensor_tensor(
                        out=st, in0=sdf, scalar=2.0, in1=hcol4[:, rt:rt + 1],
                        op0=mybir.AluOpType.mult, op1=mybir.AluOpType.add,
                    )
                    nc.vector.tensor_mul(out=outt[:, rt:rt + 1], in0=st, in1=st)
            nc.sync.dma_start(out=o_sq[:, :], in_=outt[:, :])
    return _split_excess_waits(nc)


def make_in_maps_v6(y_pred, y_true, x_values, fracture_idx, chunk_k: int = 2048,
                    group_rows: int = 16, align: int = 128, fuse_io: bool = False):
    """Sharding + group widths for v6 (strided-partition groups).

    yp/yt stay in fully sorted row order (DMA sources are contiguous row
    blocks). fcl/hcl are permuted to PARTITION order: partition p of tile rt
    holds sorted row j(p) = GR*(p % NG) + p // NG, matching the strided DMA
    dest ypt[g::NG] for group g's rows [GR*g, GR*(g+1)).
    """
    x = np.asarray(x_values, dtype=np.float32)
    dx = np.diff(x)
    if not (dx.size > 0 and bool(np.all(dx == dx[0]))):
        return None
    scale = float(0.5 * dx[0]) ** 2 / B

    y_pred = np.ascontiguousarray(np.asarray(y_pred, dtype=np.float32))
    y_true = np.ascontiguousarray(np.asarray(y_true, dtype=np.float32))
    idx = np.clip(np.asarray(fracture_idx).astype(np.int64), 0, N - 1)
    rows_all = np.arange(B)
    d_f = y_pred[rows_all, idx] - y_true[rows_all, idx]
    d_0 = y_pred[:, 0] - y_true[:, 0]
    h = (d_f - d_0).astype(np.float32)
    f = idx.astype(np.float32)

    GR = group_rows
    NG = P // GR
    # partition p <- sorted row j(p) within each 128-row tile
    p_arr = np.arange(P)
    j_of_p = GR * (p_arr % NG) + p_arr // NG
    perm_tile = np.concatenate([rt * P + j_of_p for rt in range(RT)])

    order = np.argsort(idx, kind="stable")
    in_maps = []
    fmat = np.empty((NCORES, BS), np.int64)
    for m in range(NCORES):
        rows = order[m::NCORES]
        fmat[m] = idx[rows]
        rows_p = rows[perm_tile]
        im = {
            "fcl": np.ascontiguousarray(f[rows_p].reshape(BS, 1)),
            "hcl": np.ascontiguousarray(h[rows_p].reshape(BS, 1)),
        }
        if fuse_io:
            yy = np.empty((BS, 2, N), np.float32)
            yy[:, 0, :] = y_pred[rows]
            yy[:, 1, :] = y_true[rows]
            im["yy"] = yy.reshape(BS, 2 * N)
        else:
            im["yp"] = np.ascontiguousarray(y_pred[rows])
            im["yt"] = np.ascontiguousarray(y_true[rows])
        in_maps.append(im)
    widths = []
    full_upto = []
    for rt in range(RT):
        row_w = []
        for g in range(NG):
            lo = rt * P + g * GR
            w = int(fmat[:, lo:lo + GR].max())
            w = min(N, -(-w // align) * align)
            row_w.append(w)
        widths.append(tuple(row_w))
        minf = int(fmat[:, rt * P].min())
        full_upto.append(minf // chunk_k)
    return in_maps, tuple(widths), tuple(full_upto), scale


def make_in_maps_v4(y_pred, y_true, x_values, fracture_idx, chunk_k: int = 512):
    """Sorted+striped data-parallel sharding for the ragged fast path.

    Returns (in_maps, bounds, scale) or None when dx is not uniform (caller
    falls back to the general path). Global sort rank r -> core r%8 position
    r//8, so every core sees the same f quantiles and one set of suffix
    bounds (min over cores) serves all cores' SPMD NEFF.
    """
    x = np.asarray(x_values, dtype=np.float32)
    dx = np.diff(x)
    if not (dx.size > 0 and bool(np.all(dx == dx[0]))):
        return None
    scale = float(0.5 * dx[0]) ** 2 / B

    y_pred = np.ascontiguousarray(np.asarray(y_pred, dtype=np.float32))
    y_true = np.ascontiguousarray(np.asarray(y_true, dtype=np.float32))
    idx = np.clip(np.asarray(fracture_idx).astype(np.int64), 0, N - 1)
    rows_all = np.arange(B)
    d_f = y_pred[rows_all, idx] - y_true[rows_all, idx]
    d_0 = y_pred[:, 0] - y_true[:, 0]
    h = (d_f - d_0).astype(np.float32)
    f = idx.astype(np.float32)

    order = np.argsort(idx, kind="stable")
    K2 = chunk_k
    NCH2 = N // K2
    in_maps = []
    fmat = np.empty((NCORES, BS), np.int64)
    for m in range(NCORES):
        rows = order[m::NCORES]
        fmat[m] = idx[rows]
        in_maps.append({
            "yp": np.ascontiguousarray(y_pred[rows]),
            "yt": np.ascontiguousarray(y_true[rows]),
            "fcl": np.ascontiguousarray(f[rows].reshape(BS, 1)),
            "hcl": np.ascontiguousarray(h[rows].reshape(BS, 1)),
        })
    bounds = []
    full_upto = []
    for rt in range(RT):
        row_b = []
        for c in range(NCH2):
            ps = P
            for m in range(NCORES):
                ft = fmat[m, rt * P:(rt + 1) * P]
                ps = min(ps, int(np.searchsorted(ft, c * K2, side="right")))
            row_b.append(ps)
        bounds.append(tuple(row_b))
        minf = int(fmat[:, rt * P].min())  # rows ascending -> tile min f
        full_upto.append(minf // K2)
    return in_maps, tuple(bounds), tuple(full_upto), scale


def build_nc(uniform: bool = True, reps: int = 1, io_bufs: int = 3, cmp_bufs: int = 2):
    f32 = mybir.dt.float32
    nc = bass.Bass()
    yp = nc.declare_dram_parameter("yp", [BS, N], f32, isOutput=False)
    yt = nc.declare_dram_parameter("yt", [BS, N], f32, isOutput=False)
    fcl = nc.declare_dram_parameter("fcl", [BS, 1], f32, isOutput=False)
    w = None
    if not uniform:
        w = nc.declare_dram_parameter("w", [P, N - 1], f32, isOutput=False)
    o_sq = nc.declare_dram_parameter("o_sq", [P, RT], f32, isOutput=True)

    with TileContext(nc) as tc:
        with tc.tile_pool(name="pio", bufs=io_bufs) as pio, \
             tc.tile_pool(name="pcmp", bufs=cmp_bufs) as pc, \
             tc.tile_pool(name="pers", bufs=1) as pp:
            # One-time: per-chunk f32 iota rows (values are exact ints < 2^24).
            iotas = []
            wts = []
            for c in range(NCH):
                seg = K if c < NCH - 1 else K - 1
                it = pp.tile([P, seg], f32, tag=f"iota{c}")
                nc.gpsimd.iota(
                    it, pattern=[[1, seg]], base=c * K, channel_multiplier=0,
                    allow_small_or_imprecise_dtypes=True,
                )
                iotas.append(it)
                if not uniform:
                    wt = pp.tile([P, seg], f32, tag=f"w{c}")
                    nc.sync.dma_start(out=wt, in_=w[:, c * K:c * K + seg])
                    wts.append(wt)
            outt = pp.tile([P, RT], f32, tag="outt")

            for _rep in range(reps):
                for rt in range(RT):
                    r0 = rt * P
                    fcol = pc.tile([P, 1], f32, tag="fcol")
                    nc.sync.dma_start(out=fcol, in_=fcl[r0:r0 + P, :])
                    p4 = pc.tile([P, NCH], f32, tag="p4")
                    for c in range(NCH):
                        lw = K + 1 if c < NCH - 1 else K   # load width
                        seg = lw - 1                       # segments
                        c0 = c * K
                        ypt = pio.tile([P, K + 1], f32, tag="ypt")
                        ytt = pio.tile([P, K + 1], f32, tag="ytt")
                        nc.sync.dma_start(out=ypt[:, :lw], in_=yp[r0:r0 + P, c0:c0 + lw])
                        nc.sync.dma_start(out=ytt[:, :lw], in_=yt[r0:r0 + P, c0:c0 + lw])
                        d = pc.tile([P, K + 1], f32, tag="d")
                        nc.vector.tensor_sub(out=d[:, :lw], in0=ypt[:, :lw], in1=ytt[:, :lw])
                        s = pc.tile([P, K], f32, tag="s")
                        nc.vector.tensor_add(out=s[:, :seg], in0=d[:, 0:seg], in1=d[:, 1:seg + 1])
                        src = s
                        if not uniform:
                            u = pc.tile([P, K], f32, tag="u")
                            nc.vector.tensor_mul(out=u[:, :seg], in0=s[:, :seg], in1=wts[c][:, :seg])
                            src = u
                        q = pc.tile([P, K], f32, tag="q")
                        nc.vector.scalar_tensor_tensor(
                            out=q[:, :seg], in0=iotas[c][:, :seg], scalar=fcol,
                            in1=src[:, :seg],
                            op0=mybir.AluOpType.is_lt, op1=mybir.AluOpType.mult,
                            accum_out=p4[:, c:c + 1],
                        )
                    st = pc.tile([P, 1], f32, tag="st")
                    nc.vector.tensor_reduce(
                        out=st, in_=p4, axis=mybir.AxisListType.X, op=mybir.AluOpType.add
                    )
                    nc.vector.tensor_mul(out=outt[:, rt:rt + 1], in0=st, in1=st)
            nc.sync.dma_start(out=o_sq[:, :], in_=outt[:, :])
    return _split_excess_waits(nc)


def make_in_maps(y_pred, y_true, x_values, fracture_idx):
    y_pred = np.ascontiguousarray(np.asarray(y_pred, dtype=np.float32))
    y_true = np.ascontiguousarray(np.asarray(y_true, dtype=np.float32))
    x = np.asarray(x_values, dtype=np.float32)
    idx = np.clip(np.asarray(fracture_idx).astype(np.int64), 0, N - 1)
    f = idx.astype(np.float32).reshape(B, 1)

    dx = np.diff(x)
    uniform = bool(np.all(dx == dx[0]))
    if uniform:
        scale = float(0.5 * dx[0]) ** 2 / B
    else:
        scale = 1.0 / B

    # hcl = d_f - d_0 per row (O(B) host gather; see build_nc_v3 docstring)
    rows = np.arange(B)
    d_f = y_pred[rows, idx] - y_true[rows, idx]
    d_0 = y_pred[:, 0] - y_true[:, 0]
    h = (d_f - d_0).astype(np.float32).reshape(B, 1)

    in_maps = []
    for c in range(NCORES):
        r0 = c * BS
        m = {
            "yp": y_pred[r0:r0 + BS],
            "yt": y_true[r0:r0 + BS],
            "fcl": np.ascontiguousarray(f[r0:r0 + BS]),
            "hcl": np.ascontiguousarray(h[r0:r0 + BS]),
        }
        if not uniform:
            wrow = (0.5 * dx).astype(np.float32)
            m["w"] = np.ascontiguousarray(np.broadcast_to(wrow, (P, N - 1)))
        in_maps.append(m)
    return in_maps, uniform, scale


CHUNK_K = 2048
GROUP_ROWS = 16


def kernel(y_pred, y_true, x_values, fracture_idx):
    assert y_pred.shape == (B, N), y_pred.shape
    v6 = make_in_maps_v6(y_pred, y_true, x_values, fracture_idx,
                         chunk_k=CHUNK_K, group_rows=GROUP_ROWS)
    if v6 is not None:
        in_maps, widths, full_upto, scale = v6
        key = ("v6", CHUNK_K, GROUP_ROWS, widths, full_upto)
        if key not in _nc_cache:
            _nc_cache[key] = build_nc_v6(widths, full_upto, chunk_k=CHUNK_K,
                                         group_rows=GROUP_ROWS)
        nc = _nc_cache[key]
    else:
        in_maps, uniform, scale = make_in_maps(y_pred, y_true, x_values, fracture_idx)
        key = ("main", uniform)
        if key not in _nc_cache:
            _nc_cache[key] = (
                build_nc_v3(io_bufs=3, d_bufs=1, chunk_k=4096, alt_rings=True)
                if uniform else build_nc(uniform=False)
            )
        nc = _nc_cache[key]
    res = None
    last_err = None
    for _attempt in range(3):
        try:
            res = run_bass_kernel_spmd(nc, in_maps, list(range(NCORES)))
            break
        except Exception as e:  # sporadic NRT_EXEC_UNIT_UNRECOVERABLE on this infra
            last_err = e
            try:
                import jax
                jax.clear_backends()
            except Exception:
                pass
    if res is None:
        raise last_err
    total = 0.0
    for c in range(NCORES):
        total += np.asarray(res.results[c]["o_sq"], dtype=np.float64).sum()
    return np.asarray(total * scale, dtype=np.float32)



# revision 28
# speedup vs baseline: 1.0578x; 1.0578x over previous
"""Trainium2 Bass kernel for CustomStrainEnergyLoss.

Math (d = y_pred - y_true, f = clipped fracture_idx):
    pred_int_b - true_int_b = masked_trapz(d)                 (linearity)
    t_b  = sum_j 0.5*dx_j*(d_{b,j} + d_{b,j+1}) * [j < f_b]
    out  = mean_b(t_b^2)

For the uniform grid (x_values = arange, always true for this problem), with
m1 = [i<f], m2 = [i<=f] and m2 - m1 = [i==f]:
    sum_j (d_j + d_{j+1})*m1_j = sum_i d_i*m1_i + sum_i d_i*m2_i - d_0
                               = 2*sum_i d_i*[i<f] + (d_f - d_0)
so the device does just TWO DVE passes per element (memory-bound at the
~358 GB/s per-core HBM limit):
    d = yp - yt                          (tensor_sub)
    A = sum((iota < f) * d)              (scalar_tensor_tensor, fused accum)
then S = 2A + hcol per row ([128,1] ops), out = S^2.  hcol = d_f - d_0 is an
O(B) host-side gather. The 0.5*dx weight folds into a host-side scalar on the
final mean. A general path (non-uniform dx) multiplies the trapezoid segments
by a replicated 0.5*dx row on device instead.

Sharding: pure data parallel, 512 rows per core across 8 cores, y_pred on the
sync-engine HWDGE ring and y_true on the scalar-engine ring (alternating per
chunk); per-core row results [128, 4] are squared on device, summed on host.

This neuronx-cc build rejects instructions with >1 sync wait, so
_split_excess_waits moves extra waits onto same-engine NoOps post-schedule.
"""

import numpy as np

from concourse import bass
import concourse.mybir as mybir
from concourse.tile import TileContext
from concourse.bass_utils import run_bass_kernel_spmd

B, N = 4096, 8192
NCORES = 8
BS = B // NCORES          # 512 rows per core
P = 128                   # partitions
RT = BS // P              # 4 row-tiles per core
K = 2048                  # column chunk
NCH = N // K              # 4 chunks

_nc_cache = {}


def _split_excess_waits(nc, maxw: int = 1):
    """Workaround for this neuronx-cc build: walrus codegen rejects any
    instruction carrying more than one sync wait ("Too many sync wait
    commands" in setupSyncWait). Move extra waits onto same-engine NoOps
    inserted immediately before the instruction (sequencer executes them in
    order, so semantics are unchanged)."""
    for b in nc.main_func.blocks:
        newlist = []
        for ins in b.instructions:
            si = ins.sync_info
            ow = list(si.on_wait) if si else []
            if len(ow) > maxw:
                extra, keep = ow[:len(ow) - maxw], ow[len(ow) - maxw:]
                for i in range(0, len(extra), maxw):
                    nop = mybir.InstNoOp(
                        name=nc.get_next_instruction_name(), ins=[], outs=[])
                    nop.engine = ins.engine
                    nop.sync_info = mybir.SyncInfo(
                        on_wait=list(extra[i:i + maxw]), on_update=[])
                    nc.register_instruction(nop)
                    newlist.append(nop)
                ins.sync_info = mybir.SyncInfo(
                    on_wait=list(keep), on_update=list(si.on_update))
            newlist.append(ins)
        b.instructions[:] = newlist
    return nc


def build_nc_v2(reps: int = 1, io_bufs: int = 3, cmp_bufs: int = 2):
    """Uniform-dx fast path.

    S_b = sum_i d_i*[i<f_b] + sum_i d_i*[i<=f_b] - d_0   (all over full rows)
    Per [128, 4096] chunk: one tensor_sub + two fused STT mask-reduces.
    2 MiB DMA loads, y_pred on the sync HWDGE ring, y_true on the scalar ring.
    """
    f32 = mybir.dt.float32
    K2 = 4096
    NCH2 = N // K2  # 2
    nc = bass.Bass()
    yp = nc.declare_dram_parameter("yp", [BS, N], f32, isOutput=False)
    yt = nc.declare_dram_parameter("yt", [BS, N], f32, isOutput=False)
    fcl = nc.declare_dram_parameter("fcl", [BS, 1], f32, isOutput=False)
    o_sq = nc.declare_dram_parameter("o_sq", [P, RT], f32, isOutput=True)

    with TileContext(nc) as tc:
        with tc.tile_pool(name="pio", bufs=io_bufs) as pio, \
             tc.tile_pool(name="pcmp", bufs=cmp_bufs) as pc, \
             tc.tile_pool(name="pq", bufs=1) as pq, \
             tc.tile_pool(name="pers", bufs=1) as pp:
            iotas = []
            for c in range(NCH2):
                it = pp.tile([P, K2], f32, tag=f"iota{c}")
                nc.gpsimd.iota(
                    it, pattern=[[1, K2]], base=c * K2, channel_multiplier=0,
                    allow_small_or_imprecise_dtypes=True,
                )
                iotas.append(it)
            outt = pp.tile([P, RT], f32, tag="outt")

            for _rep in range(reps):
                for rt in range(RT):
                    r0 = rt * P
                    fcol = pc.tile([P, 1], f32, tag="fcol")
                    nc.sync.dma_start(out=fcol, in_=fcl[r0:r0 + P, :])
                    pab = pc.tile([P, 2 * NCH2], f32, tag="pab")
                    d0 = pc.tile([P, 1], f32, tag="d0")
                    for c in range(NCH2):
                        c0 = c * K2
                        ypt = pio.tile([P, K2], f32, tag="ypt")
                        ytt = pio.tile([P, K2], f32, tag="ytt")
                        nc.sync.dma_start(out=ypt, in_=yp[r0:r0 + P, c0:c0 + K2])
                        nc.scalar.dma_start(out=ytt, in_=yt[r0:r0 + P, c0:c0 + K2])
                        d = pc.tile([P, K2], f32, tag="d")
                        nc.vector.tensor_sub(out=d, in0=ypt, in1=ytt)
                        if c == 0:
                            nc.vector.tensor_copy(out=d0, in_=d[:, 0:1])
                        q = pq.tile([P, K2], f32, tag="q")
                        nc.vector.scalar_tensor_tensor(
                            out=q, in0=iotas[c], scalar=fcol, in1=d,
                            op0=mybir.AluOpType.is_lt, op1=mybir.AluOpType.mult,
                            accum_out=pab[:, c:c + 1],
                        )
                        nc.vector.scalar_tensor_tensor(
                            out=q, in0=iotas[c], scalar=fcol, in1=d,
                            op0=mybir.AluOpType.is_le, op1=mybir.AluOpType.mult,
                            accum_out=pab[:, NCH2 + c:NCH2 + c + 1],
                        )
                    ssum = pc.tile([P, 1], f32, tag="ssum")
                    nc.vector.tensor_reduce(
                        out=ssum, in_=pab, axis=mybir.AxisListType.X, op=mybir.AluOpType.add
                    )
                    st = pc.tile([P, 1], f32, tag="st")
                    nc.vector.tensor_sub(out=st, in0=ssum, in1=d0)
                    nc.vector.tensor_mul(out=outt[:, rt:rt + 1], in0=st, in1=st)
            nc.sync.dma_start(out=o_sq[:, :], in_=outt[:, :])
    return _split_excess_waits(nc)


def build_nc_v3(reps: int = 1, io_bufs: int = 3, cmp_bufs: int = 2,
                chunk_k: int = 4096, d_bufs: int = 2, batched_fh: bool = True,
                alt_rings: bool = False):
    """Uniform-dx fast path, 2 DVE passes per element.

    Identity: with m1 = [i<f], m2 = [i<=f],  m2 - m1 = [i==f], so
        S_b = sum_i d_i*m1 + sum_i d_i*m2 - d_0 = 2*sum_i d_i*[i<f] + (d_f - d_0).
    The host supplies hcol = d_f - d_0 per row (an O(B) gather); the device
    does d = yp - yt and ONE fused mask-reduce per chunk, then
    S = 2*A + hcol, out = S^2.
    """
    f32 = mybir.dt.float32
    K2 = chunk_k
    NCH2 = N // K2
    nc = bass.Bass()
    yp = nc.declare_dram_parameter("yp", [BS, N], f32, isOutput=False)
    yt = nc.declare_dram_parameter("yt", [BS, N], f32, isOutput=False)
    fcl = nc.declare_dram_parameter("fcl", [BS, 1], f32, isOutput=False)
    hcl = nc.declare_dram_parameter("hcl", [BS, 1], f32, isOutput=False)
    o_sq = nc.declare_dram_parameter("o_sq", [P, RT], f32, isOutput=True)
    # [512,1] viewed as [128, RT]: column rt holds rows rt*128..rt*128+127
    fview = fcl.rearrange("(rt p) one -> p (rt one)", p=P)
    hview = hcl.rearrange("(rt p) one -> p (rt one)", p=P)

    with TileContext(nc) as tc:
        with tc.tile_pool(name="pio", bufs=io_bufs) as pio, \
             tc.tile_pool(name="pcmp", bufs=cmp_bufs) as pc, \
             tc.tile_pool(name="pd", bufs=d_bufs) as pd, \
             tc.tile_pool(name="pq", bufs=1) as pq, \
             tc.tile_pool(name="pers", bufs=1) as pp:
            iotas = []
            for c in range(NCH2):
                it = pp.tile([P, K2], f32, tag=f"iota{c}")
                nc.gpsimd.iota(
                    it, pattern=[[1, K2]], base=c * K2, channel_multiplier=0,
                    allow_small_or_imprecise_dtypes=True,
                )
                iotas.append(it)
            outt = pp.tile([P, RT], f32, tag="outt")

            for _rep in range(reps):
                if batched_fh:
                    fcol4 = pc.tile([P, RT], f32, tag="fcol4")
                    nc.sync.dma_start(out=fcol4, in_=fview)
                    hcol4 = pc.tile([P, RT], f32, tag="hcol4")
                    nc.sync.dma_start(out=hcol4, in_=hview)
                for rt in range(RT):
                    r0 = rt * P
                    if not batched_fh:
                        fcol4 = pc.tile([P, RT], f32, tag="fcol4")
                        nc.sync.dma_start(out=fcol4[:, rt:rt + 1], in_=fcl[r0:r0 + P, :])
                        hcol4 = pc.tile([P, RT], f32, tag="hcol4")
                        nc.sync.dma_start(out=hcol4[:, rt:rt + 1], in_=hcl[r0:r0 + P, :])
                    pab = pc.tile([P, NCH2], f32, tag="pab")
                    for c in range(NCH2):
                        c0 = c * K2
                        ypt = pio.tile([P, K2], f32, tag="ypt")
                        ytt = pio.tile([P, K2], f32, tag="ytt")
                        e0, e1 = (nc.sync, nc.scalar)
                        if alt_rings and (rt * NCH2 + c) % 2 == 1:
                            e0, e1 = (nc.scalar, nc.sync)
                        e0.dma_start(out=ypt, in_=yp[r0:r0 + P, c0:c0 + K2])
                        e1.dma_start(out=ytt, in_=yt[r0:r0 + P, c0:c0 + K2])
                        d = pd.tile([P, K2], f32, tag="d")
                        nc.vector.tensor_sub(out=d, in0=ypt, in1=ytt)
                        q = pq.tile([P, K2], f32, tag="q")
                        nc.vector.scalar_tensor_tensor(
                            out=q, in0=iotas[c], scalar=fcol4[:, rt:rt + 1], in1=d,
                            op0=mybir.AluOpType.is_lt, op1=mybir.AluOpType.mult,
                            accum_out=pab[:, c:c + 1],
                        )
                    ssum = pc.tile([P, 1], f32, tag="ssum")
                    if NCH2 > 1:
                        nc.vector.tensor_reduce(
                            out=ssum, in_=pab, axis=mybir.AxisListType.X,
                            op=mybir.AluOpType.add,
                        )
                    else:
                        ssum = pab
                    st = pc.tile([P, 1], f32, tag="st")
                    nc.vector.scalar_tensor_tensor(
                        out=st, in0=ssum, scalar=2.0, in1=hcol4[:, rt:rt + 1],
                        op0=mybir.AluOpType.mult, op1=mybir.AluOpType.add,
                    )
                    nc.vector.tensor_mul(out=outt[:, rt:rt + 1], in0=st, in1=st)
            nc.sync.dma_start(out=o_sq[:, :], in_=outt[:, :])
    return _split_excess_waits(nc)


def build_nc_v4(bounds, chunk_k: int = 512, reps: int = 1, io_bufs: int = 4,
                cmp_bufs: int = 2, d_bufs: int = 2, full_upto=None,
                do_dma: bool = True, do_compute: bool = True):
    """Ragged fast path: rows are pre-sorted by fracture_idx on host, so for
    column chunk c only the partition-suffix of rows with f > c*K contributes;
    everything below the suffix is masked to exactly 0 anyway. We skip both
    the DMA and the vector work for those rows: traffic drops from B*N to
    ~sum(f) elements (~2x for uniform f). `bounds[rt][c]` is the first
    partition that needs chunk c in row-tile rt (shared across cores = min
    over cores, so one SPMD NEFF serves all 8).
    Per-tile accumulator pab is memset to 0, so skipped (row, chunk) cells
    contribute nothing; loaded-but-masked rows contribute (iota<f)*d = 0.
    """
    f32 = mybir.dt.float32
    K2 = chunk_k
    NCH2 = N // K2
    nc = bass.Bass()
    yp = nc.declare_dram_parameter("yp", [BS, N], f32, isOutput=False)
    yt = nc.declare_dram_parameter("yt", [BS, N], f32, isOutput=False)
    fcl = nc.declare_dram_parameter("fcl", [BS, 1], f32, isOutput=False)
    hcl = nc.declare_dram_parameter("hcl", [BS, 1], f32, isOutput=False)
    o_sq = nc.declare_dram_parameter("o_sq", [P, RT], f32, isOutput=True)
    fview = fcl.rearrange("(rt p) one -> p (rt one)", p=P)
    hview = hcl.rearrange("(rt p) one -> p (rt one)", p=P)

    with TileContext(nc) as tc:
        with tc.tile_pool(name="pio", bufs=io_bufs) as pio, \
             tc.tile_pool(name="pcmp", bufs=cmp_bufs) as pc, \
             tc.tile_pool(name="pd", bufs=d_bufs) as pd, \
             tc.tile_pool(name="pq", bufs=1) as pq, \
             tc.tile_pool(name="pers", bufs=1) as pp:
            iota = pp.tile([P, N], f32, tag="iota")
            nc.gpsimd.iota(
                iota, pattern=[[1, N]], base=0, channel_multiplier=0,
                allow_small_or_imprecise_dtypes=True,
            )
            outt = pp.tile([P, RT], f32, tag="outt")

            # Compute APs must start on a 32-partition quadrant boundary, so
            # vector ops below run on [ps32:P) while DMA fills only [ps:P).
            # Rows in [ps32, ps) are masked to 0 by (iota < f); memset every
            # io buffer once so their stale contents are finite (0*x == 0).
            for _i in range(io_bufs):
                zyp = pio.tile([P, K2], f32, tag="ypt")
                nc.vector.memset(zyp, 0.0)
                zyt = pio.tile([P, K2], f32, tag="ytt")
                nc.vector.memset(zyt, 0.0)

            flip = 0
            for _rep in range(reps):
                fcol4 = pc.tile([P, RT], f32, tag="fcol4")
                nc.sync.dma_start(out=fcol4, in_=fview)
                hcol4 = pc.tile([P, RT], f32, tag="hcol4")
                nc.sync.dma_start(out=hcol4, in_=hview)
                for rt in range(RT):
                    r0 = rt * P
                    pab = pc.tile([P, NCH2], f32, tag="pab")
                    nc.vector.memset(pab, 0.0)
                    for c in range(NCH2):
                        ps = bounds[rt][c]
                        if ps >= P:
                            break  # f sorted -> later chunks needed by nobody
                        c0 = c * K2
                        ypt = pio.tile([P, K2], f32, tag="ypt")
                        ytt = pio.tile([P, K2], f32, tag="ytt")
                        e0, e1 = (nc.sync, nc.scalar) if flip == 0 else (nc.scalar, nc.sync)
                        flip ^= 1
                        if do_dma:
                            e0.dma_start(out=ypt[ps:P, :], in_=yp[r0 + ps:r0 + P, c0:c0 + K2])
                            e1.dma_start(out=ytt[ps:P, :], in_=yt[r0 + ps:r0 + P, c0:c0 + K2])
                        if not do_compute:
                            continue
                        # Compute on all 128 partitions (nonzero partition
                        # starts are limited to one 32-quadrant); rows below
                        # ps hold stale-but-finite data and mask to 0.
                        d = pd.tile([P, K2], f32, tag="d")
                        if full_upto is not None and c < full_upto[rt]:
                            # Chunk lies inside every row's prefix: fuse
                            # subtract + row-sum in one DVE pass, no mask.
                            nc.vector.tensor_tensor_reduce(
                                out=d, in0=ypt, in1=ytt, scale=1.0, scalar=0.0,
                                op0=mybir.AluOpType.subtract,
                                op1=mybir.AluOpType.add,
                                accum_out=pab[:, c:c + 1],
                            )
                            continue
                        nc.vector.tensor_sub(out=d, in0=ypt, in1=ytt)
                        q = pq.tile([P, K2], f32, tag="q")
                        nc.vector.scalar_tensor_tensor(
                            out=q, in0=iota[:, c0:c0 + K2],
                            scalar=fcol4[:, rt:rt + 1], in1=d,
                            op0=mybir.AluOpType.is_lt, op1=mybir.AluOpType.mult,
                            accum_out=pab[:, c:c + 1],
                        )
                    ssum = pc.tile([P, 1], f32, tag="ssum")
                    nc.vector.tensor_reduce(
                        out=ssum, in_=pab, axis=mybir.AxisListType.X,
                        op=mybir.AluOpType.add,
                    )
                    st = pc.tile([P, 1], f32, tag="st")
                    nc.vector.scalar_tensor_tensor(
                        out=st, in0=ssum, scalar=2.0, in1=hcol4[:, rt:rt + 1],
                        op0=mybir.AluOpType.mult, op1=mybir.AluOpType.add,
                    )
                    nc.vector.tensor_mul(out=outt[:, rt:rt + 1], in0=st, in1=st)
            nc.sync.dma_start(out=o_sq[:, :], in_=outt[:, :])
    return _split_excess_waits(nc)


def build_nc_v5(widths, full_upto, chunk_k: int = 2048, reps: int = 1,
                io_bufs: int = 2, cmp_bufs: int = 2, d_bufs: int = 1,
                group_rows: int = 8, do_dma: bool = True,
                do_compute: bool = True):
    """Ragged fast path, row-group full-prefix DMA.

    v4's [row-suffix x 512-col] DMAs made 2 KiB descriptors and died on
    per-descriptor overhead (250 GB/s dma-only). Here each DMA loads
    [group_rows x W_g] where W_g = the group's shared prefix width, so every
    descriptor is one FULL row prefix (up to 32 KiB). widths[rt][g] (elems,
    nondecreasing in g) and full_upto[rt] (chunks fully inside every row's
    prefix) are host-computed from sorted f, shared across cores (max over
    cores) so one SPMD NEFF serves all 8.
    Full chunks need no mask: row-sums of yp and yt accumulate separately
    (single-input tensor_reduce; tensor_tensor_reduce trips "ISA wrong
    length" in this neuronx-cc build) and subtract at the end. Boundary
    chunks use sub + masked STT. Rows keep real data on [0, W_g); columns
    beyond W_g hold stale-but-finite values (prologue memset) that the
    (iota < f) mask zeroes.
    """
    f32 = mybir.dt.float32
    K2 = chunk_k
    GR = group_rows
    NG = P // GR
    nc = bass.Bass()
    yp = nc.declare_dram_parameter("yp", [BS, N], f32, isOutput=False)
    yt = nc.declare_dram_parameter("yt", [BS, N], f32, isOutput=False)
    fcl = nc.declare_dram_parameter("fcl", [BS, 1], f32, isOutput=False)
    hcl = nc.declare_dram_parameter("hcl", [BS, 1], f32, isOutput=False)
    o_sq = nc.declare_dram_parameter("o_sq", [P, RT], f32, isOutput=True)
    fview = fcl.rearrange("(rt p) one -> p (rt one)", p=P)
    hview = hcl.rearrange("(rt p) one -> p (rt one)", p=P)

    wmax = [max(widths[rt]) for rt in range(RT)]

    with TileContext(nc) as tc:
        with tc.tile_pool(name="pio", bufs=io_bufs) as pio, \
             tc.tile_pool(name="pcmp", bufs=cmp_bufs) as pc, \
             tc.tile_pool(name="pd", bufs=d_bufs) as pd, \
             tc.tile_pool(name="pq", bufs=1) as pq, \
             tc.tile_pool(name="pers", bufs=1) as pp:
            iota = pp.tile([P, N], f32, tag="iota")
            nc.gpsimd.iota(
                iota, pattern=[[1, N]], base=0, channel_multiplier=0,
                allow_small_or_imprecise_dtypes=True,
            )
            outt = pp.tile([P, RT], f32, tag="outt")
            nc.vector.memset(outt, 0.0)

            for _i in range(io_bufs):
                zyp = pio.tile([P, N], f32, tag="ypt")
                nc.vector.memset(zyp, 0.0)
                zyt = pio.tile([P, N], f32, tag="ytt")
                nc.vector.memset(zyt, 0.0)

            flip = 0
            for _rep in range(reps):
                fcol4 = pc.tile([P, RT], f32, tag="fcol4")
                nc.sync.dma_start(out=fcol4, in_=fview)
                hcol4 = pc.tile([P, RT], f32, tag="hcol4")
                nc.sync.dma_start(out=hcol4, in_=hview)
                for rt in range(RT):
                    r0 = rt * P
                    wt = wmax[rt]
                    nch_t = (wt + K2 - 1) // K2
                    ypt = pio.tile([P, N], f32, tag="ypt")
                    ytt = pio.tile([P, N], f32, tag="ytt")
                    if do_dma:
                        for g in range(NG):
                            wg = widths[rt][g]
                            if wg <= 0:
                                continue
                            p0 = g * GR
                            e0, e1 = (nc.sync, nc.scalar) if flip == 0 else (nc.scalar, nc.sync)
                            flip ^= 1
                            e0.dma_start(out=ypt[p0:p0 + GR, :wg],
                                         in_=yp[r0 + p0:r0 + p0 + GR, :wg])
                            e1.dma_start(out=ytt[p0:p0 + GR, :wg],
                                         in_=yt[r0 + p0:r0 + p0 + GR, :wg])
                    if not do_compute:
                        continue
                    # pabp accumulates +sum(yp)/-masked contributions per
                    # chunk column; pabt the matching yt sums (full chunks
                    # only; boundary chunks fold the whole (yp-yt) masked
                    # sum into pabp and leave pabt's column at 0).
                    pabp = pc.tile([P, N // K2], f32, tag="pabp")
                    nc.vector.memset(pabp, 0.0)
                    pabt = pc.tile([P, N // K2], f32, tag="pabt")
                    nc.vector.memset(pabt, 0.0)
                    for c in range(nch_t):
                        c0 = c * K2
                        w = min(K2, wt - c0)
                        if c < full_upto[rt]:
                            nc.vector.tensor_reduce(
                                out=pabp[:, c:c + 1], in_=ypt[:, c0:c0 + w],
                                axis=mybir.AxisListType.X, op=mybir.AluOpType.add,
                            )
                            nc.vector.tensor_reduce(
                                out=pabt[:, c:c + 1], in_=ytt[:, c0:c0 + w],
                                axis=mybir.AxisListType.X, op=mybir.AluOpType.add,
                            )
                            continue
                        d = pd.tile([P, K2], f32, tag="d")
                        nc.vector.tensor_sub(out=d[:, :w], in0=ypt[:, c0:c0 + w],
                                             in1=ytt[:, c0:c0 + w])
                        q = pq.tile([P, K2], f32, tag="q")
                        nc.vector.scalar_tensor_tensor(
                            out=q[:, :w], in0=iota[:, c0:c0 + w],
                            scalar=fcol4[:, rt:rt + 1], in1=d[:, :w],
                            op0=mybir.AluOpType.is_lt, op1=mybir.AluOpType.mult,
                            accum_out=pabp[:, c:c + 1],
                        )
                    ssp = pc.tile([P, 1], f32, tag="ssp")
                    nc.vector.tensor_reduce(
                        out=ssp, in_=pabp, axis=mybir.AxisListType.X,
                        op=mybir.AluOpType.add,
                    )
                    sst = pc.tile([P, 1], f32, tag="sst")
                    nc.vector.tensor_reduce(
                        out=sst, in_=pabt, axis=mybir.AxisListType.X,
                        op=mybir.AluOpType.add,
                    )
                    sdf = pc.tile([P, 1], f32, tag="sdf")
                    nc.vector.tensor_sub(out=sdf, in0=ssp, in1=sst)
                    st = pc.tile([P, 1], f32, tag="st")
                    nc.vector.scalar_tensor_tensor(
                        out=st, in0=sdf, scalar=2.0, in1=hcol4[:, rt:rt + 1],
                        op0=mybir.AluOpType.mult, op1=mybir.AluOpType.add,
                    )
                    nc.vector.tensor_mul(out=outt[:, rt:rt + 1], in0=st, in1=st)
            nc.sync.dma_start(out=o_sq[:, :], in_=outt[:, :])
    return _split_excess_waits(nc)


def make_in_maps_v5(y_pred, y_true, x_values, fracture_idx, chunk_k: int = 2048,
                    group_rows: int = 8, align: int = 128):
    """Sharding + group widths for v5. Returns None when dx non-uniform."""
    x = np.asarray(x_values, dtype=np.float32)
    dx = np.diff(x)
    if not (dx.size > 0 and bool(np.all(dx == dx[0]))):
        return None
    scale = float(0.5 * dx[0]) ** 2 / B

    y_pred = np.ascontiguousarray(np.asarray(y_pred, dtype=np.float32))
    y_true = np.ascontiguousarray(np.asarray(y_true, dtype=np.float32))
    idx = np.clip(np.asarray(fracture_idx).astype(np.int64), 0, N - 1)
    rows_all = np.arange(B)
    d_f = y_pred[rows_all, idx] - y_true[rows_all, idx]
    d_0 = y_pred[:, 0] - y_true[:, 0]
    h = (d_f - d_0).astype(np.float32)
    f = idx.astype(np.float32)

    order = np.argsort(idx, kind="stable")
    in_maps = []
    fmat = np.empty((NCORES, BS), np.int64)
    for m in range(NCORES):
        rows = order[m::NCORES]
        fmat[m] = idx[rows]
        in_maps.append({
            "yp": np.ascontiguousarray(y_pred[rows]),
            "yt": np.ascontiguousarray(y_true[rows]),
            "fcl": np.ascontiguousarray(f[rows].reshape(BS, 1)),
            "hcl": np.ascontiguousarray(h[rows].reshape(BS, 1)),
        })
    GR = group_rows
    NG = P // GR
    widths = []
    full_upto = []
    for rt in range(RT):
        row_w = []
        for g in range(NG):
            lo = rt * P + g * GR
            w = int(fmat[:, lo:lo + GR].max())  # max f over group, all cores
            w = min(N, -(-w // align) * align)
            row_w.append(w)
        widths.append(tuple(row_w))
        minf = int(fmat[:, rt * P].min())
        full_upto.append(minf // chunk_k)
    return in_maps, tuple(widths), tuple(full_upto), scale


def build_nc_v6(widths, full_upto, chunk_k: int = 2048, reps: int = 1,
                io_bufs: int = 2, cmp_bufs: int = 2, d_bufs: int = 1,
                group_rows: int = 16, nqueues: int = 2,
                do_dma: bool = True, do_compute: bool = True,
                act_full: bool = False, gp_sub: bool = False,
                fh_gp: bool = False, fuse_io: bool = False):
    """Ragged fast path, engine-spread row-group DMA.

    Per-dma_start throughput is (partition span / 8) x ~27 GB/s SDMA engines
    (measured: 8-row groups 80 GB/s, 32-row 232 GB/s, 128-row ~360 GB/s), so
    v6 strides each group's GR rows across partitions g, g+NG, g+2*NG, ...
    (NG = 128/GR): every group DMA then touches all 16 SDMA engines at full
    rate while keeping the fine per-group prefix widths (~2% overshoot).
    DRAM stays row-sorted and contiguous; fcl/hcl are host-permuted to
    partition order, so compute is identical to v5.
    """
    f32 = mybir.dt.float32
    K2 = chunk_k
    GR = group_rows
    NG = P // GR
    nc = bass.Bass()
    if fuse_io:
        yy = nc.declare_dram_parameter("yy", [BS, 2 * N], f32, isOutput=False)
        yy3 = yy.rearrange("b (t n) -> b t n", t=2)
    else:
        yp = nc.declare_dram_parameter("yp", [BS, N], f32, isOutput=False)
        yt = nc.declare_dram_parameter("yt", [BS, N], f32, isOutput=False)
    fcl = nc.declare_dram_parameter("fcl", [BS, 1], f32, isOutput=False)
    hcl = nc.declare_dram_parameter("hcl", [BS, 1], f32, isOutput=False)
    o_sq = nc.declare_dram_parameter("o_sq", [P, RT], f32, isOutput=True)
    fview = fcl.rearrange("(rt p) one -> p (rt one)", p=P)
    hview = hcl.rearrange("(rt p) one -> p (rt one)", p=P)

    wmax = [max(widths[rt]) for rt in range(RT)]
    queues = [nc.sync, nc.scalar, nc.gpsimd][:nqueues]

    with TileContext(nc) as tc:
        with tc.tile_pool(name="pio", bufs=io_bufs) as pio, \
             tc.tile_pool(name="pcmp", bufs=cmp_bufs) as pc, \
             tc.tile_pool(name="pd", bufs=d_bufs) as pd, \
             tc.tile_pool(name="pq", bufs=1) as pq, \
             tc.tile_pool(name="pers", bufs=1) as pp:
            iota = pp.tile([P, N], f32, tag="iota")
            nc.gpsimd.iota(
                iota, pattern=[[1, N]], base=0, channel_multiplier=0,
                allow_small_or_imprecise_dtypes=True,
            )
            outt = pp.tile([P, RT], f32, tag="outt")
            nc.vector.memset(outt, 0.0)

            if fuse_io:
                for _i in range(io_bufs):
                    zyy = pio.tile([P, 2 * N], f32, tag="yyt")
                    nc.vector.memset(zyy, 0.0)
            else:
                for _i in range(io_bufs):
                    zyp = pio.tile([P, N], f32, tag="ypt")
                    nc.vector.memset(zyp, 0.0)
                    zyt = pio.tile([P, N], f32, tag="ytt")
                    nc.vector.memset(zyt, 0.0)

            qi = 0
            for _rep in range(reps):
                fq = nc.gpsimd if fh_gp else nc.sync
                fcol4 = pc.tile([P, RT], f32, tag="fcol4")
                fq.dma_start(out=fcol4, in_=fview)
                hcol4 = pc.tile([P, RT], f32, tag="hcol4")
                fq.dma_start(out=hcol4, in_=hview)
                for rt in range(RT):
                    r0 = rt * P
                    wt = wmax[rt]
                    nch_t = (wt + K2 - 1) // K2
                    if fuse_io:
                        yyt = pio.tile([P, 2 * N], f32, tag="yyt")
                        yyt3 = yyt.rearrange("p (t n) -> p t n", t=2)
                        ypt = yyt[:, 0:N]
                        ytt = yyt[:, N:2 * N]
                        if do_dma:
                            for g in range(NG):
                                wg = widths[rt][g]
                                if wg <= 0:
                                    continue
                                s0 = r0 + GR * g
                                queues[qi % nqueues].dma_start(
                                    out=yyt3[g::NG, :, :wg],
                                    in_=yy3[s0:s0 + GR, :, :wg])
                                qi += 1
                    else:
                        ypt = pio.tile([P, N], f32, tag="ypt")
                        ytt = pio.tile([P, N], f32, tag="ytt")
                        if do_dma:
                            for g in range(NG):
                                wg = widths[rt][g]
                                if wg <= 0:
                                    continue
                                s0 = r0 + GR * g
                                queues[qi % nqueues].dma_start(
                                    out=ypt[g::NG, :wg], in_=yp[s0:s0 + GR, :wg])
                                queues[(qi + 1) % nqueues].dma_start(
                                    out=ytt[g::NG, :wg], in_=yt[s0:s0 + GR, :wg])
                                qi += 2
                    if not do_compute:
                        continue
                    pabp = pc.tile([P, N // K2], f32, tag="pabp")
                    nc.vector.memset(pabp, 0.0)
                    pabt = pc.tile([P, N // K2], f32, tag="pabt")
                    nc.vector.memset(pabt, 0.0)
                    for c in range(nch_t):
                        c0 = c * K2
                        w = min(K2, wt - c0)
                        if c < full_upto[rt]:
                            if act_full:
                                ascr = pq.tile([P, K2], f32, tag="ascr")
                                nc.scalar.activation(
                                    out=ascr[:, :w], in_=ypt[:, c0:c0 + w],
                                    func=mybir.ActivationFunctionType.Copy,
                                    accum_out=pabp[:, c:c + 1],
                                )
                                nc.scalar.activation(
                                    out=ascr[:, :w], in_=ytt[:, c0:c0 + w],
                                    func=mybir.ActivationFunctionType.Copy,
                                    accum_out=pabt[:, c:c + 1],
                                )
                                continue
                            nc.vector.tensor_reduce(
                                out=pabp[:, c:c + 1], in_=ypt[:, c0:c0 + w],
                                axis=mybir.AxisListType.X, op=mybir.AluOpType.add,
                            )
                            nc.vector.tensor_reduce(
                                out=pabt[:, c:c + 1], in_=ytt[:, c0:c0 + w],
                                axis=mybir.AxisListType.X, op=mybir.AluOpType.add,
                            )
                            continue
                        d = pd.tile([P, K2], f32, tag="d")
                        sub_eng = nc.gpsimd if gp_sub else nc.vector
                        sub_eng.tensor_sub(out=d[:, :w], in0=ypt[:, c0:c0 + w],
                                           in1=ytt[:, c0:c0 + w])
                        q = pq.tile([P, K2], f32, tag="q")
                        nc.vector.scalar_tensor_tensor(
                            out=q[:, :w], in0=iota[:, c0:c0 + w],
                            scalar=fcol4[:, rt:rt + 1], in1=d[:, :w],
                            op0=mybir.AluOpType.is_lt, op1=mybir.AluOpType.mult,
                            accum_out=pabp[:, c:c + 1],
                        )
                    ssp = pc.tile([P, 1], f32, tag="ssp")
                    nc.vector.tensor_reduce(
                        out=ssp, in_=pabp, axis=mybir.AxisListType.X,
                        op=mybir.AluOpType.add,
                    )
                    sst = pc.tile([P, 1], f32, tag="sst")
                    nc.vector.tensor_reduce(
                        out=sst, in_=pabt, axis=mybir.AxisListType.X,
                        op=mybir.AluOpType.add,
                    )
                    sdf = pc.tile([P, 1], f32, tag="sdf")
                    nc.vector.tensor_sub(out=sdf, in0=ssp, in1=sst)
                    st = pc.tile([P, 1], f32, tag="st")
                    nc.vector.scalar_tensor_tensor(
                        out=st, in0=sdf, scalar=2.0, in1=hcol4[:, rt:rt + 1],
                        op0=mybir.AluOpType.mult, op1=mybir.AluOpType.add,
                    )
                    nc.vector.tensor_mul(out=outt[:, rt:rt + 1], in0=st, in1=st)
            nc.sync.dma_start(out=o_sq[:, :], in_=outt[:, :])
    return _split_excess_waits(nc)


def make_in_maps_v6(y_pred, y_true, x_values, fracture_idx, chunk_k: int = 2048,
                    group_rows: int = 16, align: int = 128, fuse_io: bool = False):
    """Sharding + group widths for v6 (strided-partition groups).

    yp/yt stay in fully sorted row order (DMA sources are contiguous row
    blocks). fcl/hcl are permuted to PARTITION order: partition p of tile rt
    holds sorted row j(p) = GR*(p % NG) + p // NG, matching the strided DMA
    dest ypt[g::NG] for group g's rows [GR*g, GR*(g+1)).
    """
    x = np.asarray(x_values, dtype=np.float32)
    dx = np.diff(x)
    if not (dx.size > 0 and bool(np.all(dx == dx[0]))):
        return None
    scale = float(0.5 * dx[0]) ** 2 / B

    y_pred = np.ascontiguousarray(np.asarray(y_pred, dtype=np.float32))
    y_true = np.ascontiguousarray(np.asarray(y_true, dtype=np.float32))
    idx = np.clip(np.asarray(fracture_idx).astype(np.int64), 0, N - 1)
    rows_all = np.arange(B)
    d_f = y_pred[rows_all, idx] - y_true[rows_all, idx]
    d_0 = y_pred[:, 0] - y_true[:, 0]
    h = (d_f - d_0).astype(np.float32)
    f = idx.astype(np.float32)

    GR = group_rows
    NG = P // GR
    # partition p <- sorted row j(p) within each 128-row tile
    p_arr = np.arange(P)
    j_of_p = GR * (p_arr % NG) + p_arr // NG
    perm_tile = np.concatenate([rt * P + j_of_p for rt in range(RT)])

    order = np.argsort(idx, kind="stable")
    in_maps = []
    fmat = np.empty((NCORES, BS), np.int64)
    for m in range(NCORES):
        rows = order[m::NCORES]
        fmat[m] = idx[rows]
        rows_p = rows[perm_tile]
        im = {
            "fcl": np.ascontiguousarray(f[rows_p].reshape(BS, 1)),
            "hcl": np.ascontiguousarray(h[rows_p].reshape(BS, 1)),
        }
        if fuse_io:
            yy = np.empty((BS, 2, N), np.float32)
            yy[:, 0, :] = y_pred[rows]
            yy[:, 1, :] = y_true[rows]
            im["yy"] = yy.reshape(BS, 2 * N)
        else:
            im["yp"] = np.ascontiguousarray(y_pred[rows])
            im["yt"] = np.ascontiguousarray(y_true[rows])
        in_maps.append(im)
    widths = []
    full_upto = []
    for rt in range(RT):
        row_w = []
        for g in range(NG):
            lo = rt * P + g * GR
            w = int(fmat[:, lo:lo + GR].max())
            w = min(N, -(-w // align) * align)
            row_w.append(w)
        widths.append(tuple(row_w))
        minf = int(fmat[:, rt * P].min())
        full_upto.append(minf // chunk_k)
    return in_maps, tuple(widths), tuple(full_upto), scale


def make_in_maps_v4(y_pred, y_true, x_values, fracture_idx, chunk_k: int = 512):
    """Sorted+striped data-parallel sharding for the ragged fast path.

    Returns (in_maps, bounds, scale) or None when dx is not uniform (caller
    falls back to the general path). Global sort rank r -> core r%8 position
    r//8, so every core sees the same f quantiles and one set of suffix
    bounds (min over cores) serves all cores' SPMD NEFF.
    """
    x = np.asarray(x_values, dtype=np.float32)
    dx = np.diff(x)
    if not (dx.size > 0 and bool(np.all(dx == dx[0]))):
        return None
    scale = float(0.5 * dx[0]) ** 2 / B

    y_pred = np.ascontiguousarray(np.asarray(y_pred, dtype=np.float32))
    y_true = np.ascontiguousarray(np.asarray(y_true, dtype=np.float32))
    idx = np.clip(np.asarray(fracture_idx).astype(np.int64), 0, N - 1)
    rows_all = np.arange(B)
    d_f = y_pred[rows_all, idx] - y_true[rows_all, idx]
    d_0 = y_pred[:, 0] - y_true[:, 0]
    h = (d_f - d_0).astype(np.float32)
    f = idx.astype(np.float32)

    order = np.argsort(idx, kind="stable")
    K2 = chunk_k
    NCH2 = N // K2
    in_maps = []
    fmat = np.empty((NCORES, BS), np.int64)
    for m in range(NCORES):
        rows = order[m::NCORES]
        fmat[m] = idx[rows]
        in_maps.append({
            "yp": np.ascontiguousarray(y_pred[rows]),
            "yt": np.ascontiguousarray(y_true[rows]),
            "fcl": np.ascontiguousarray(f[rows].reshape(BS, 1)),
            "hcl": np.ascontiguousarray(h[rows].reshape(BS, 1)),
        })
    bounds = []
    full_upto = []
    for rt in range(RT):
        row_b = []
        for c in range(NCH2):
            ps = P
            for m in range(NCORES):
                ft = fmat[m, rt * P:(rt + 1) * P]
                ps = min(ps, int(np.searchsorted(ft, c * K2, side="right")))
            row_b.append(ps)
        bounds.append(tuple(row_b))
        minf = int(fmat[:, rt * P].min())  # rows ascending -> tile min f
        full_upto.append(minf // K2)
    return in_maps, tuple(bounds), tuple(full_upto), scale


def build_nc(uniform: bool = True, reps: int = 1, io_bufs: int = 3, cmp_bufs: int = 2):
    f32 = mybir.dt.float32
    nc = bass.Bass()
    yp = nc.declare_dram_parameter("yp", [BS, N], f32, isOutput=False)
    yt = nc.declare_dram_parameter("yt", [BS, N], f32, isOutput=False)
    fcl = nc.declare_dram_parameter("fcl", [BS, 1], f32, isOutput=False)
    w = None
    if not uniform:
        w = nc.declare_dram_parameter("w", [P, N - 1], f32, isOutput=False)
    o_sq = nc.declare_dram_parameter("o_sq", [P, RT], f32, isOutput=True)

    with TileContext(nc) as tc:
        with tc.tile_pool(name="pio", bufs=io_bufs) as pio, \
             tc.tile_pool(name="pcmp", bufs=cmp_bufs) as pc, \
             tc.tile_pool(name="pers", bufs=1) as pp:
            # One-time: per-chunk f32 iota rows (values are exact ints < 2^24).
            iotas = []
            wts = []
            for c in range(NCH):
                seg = K if c < NCH - 1 else K - 1
                it = pp.tile([P, seg], f32, tag=f"iota{c}")
                nc.gpsimd.iota(
                    it, pattern=[[1, seg]], base=c * K, channel_multiplier=0,
                    allow_small_or_imprecise_dtypes=True,
                )
                iotas.append(it)
                if not uniform:
                    wt = pp.tile([P, seg], f32, tag=f"w{c}")
                    nc.sync.dma_start(out=wt, in_=w[:, c * K:c * K + seg])
                    wts.append(wt)
            outt = pp.tile([P, RT], f32, tag="outt")

            for _rep in range(reps):
                for rt in range(RT):
                    r0 = rt * P
                    fcol = pc.tile([P, 1], f32, tag="fcol")
                    nc.sync.dma_start(out=fcol, in_=fcl[r0:r0 + P, :])
                    p4 = pc.tile([P, NCH], f32, tag="p4")
                    for c in range(NCH):
                        lw = K + 1 if c < NCH - 1 else K   # load width
                        seg = lw - 1                       # segments
                        c0 = c * K
                        ypt = pio.tile([P, K + 1], f32, tag="ypt")
                        ytt = pio.tile([P, K + 1], f32, tag="ytt")
                        nc.sync.dma_start(out=ypt[:, :lw], in_=yp[r0:r0 + P, c0:c0 + lw])
                        nc.sync.dma_start(out=ytt[:, :lw], in_=yt[r0:r0 + P, c0:c0 + lw])
                        d = pc.tile([P, K + 1], f32, tag="d")
                        nc.vector.tensor_sub(out=d[:, :lw], in0=ypt[:, :lw], in1=ytt[:, :lw])
                        s = pc.tile([P, K], f32, tag="s")
                        nc.vector.tensor_add(out=s[:, :seg], in0=d[:, 0:seg], in1=d[:, 1:seg + 1])
                        src = s
                        if not uniform:
                            u = pc.tile([P, K], f32, tag="u")
                            nc.vector.tensor_mul(out=u[:, :seg], in0=s[:, :seg], in1=wts[c][:, :seg])
                            src = u
                        q = pc.tile([P, K], f32, tag="q")
                        nc.vector.scalar_tensor_tensor(
                            out=q[:, :seg], in0=iotas[c][:, :seg], scalar=fcol,
                            in1=src[:, :seg],
                            op0=mybir.AluOpType.is_lt, op1=mybir.AluOpType.mult,
                            accum_out=p4[:, c:c + 1],
                        )
                    st = pc.tile([P, 1], f32, tag="st")
                    nc.vector.tensor_reduce(
                        out=st, in_=p4, axis=mybir.AxisListType.X, op=mybir.AluOpType.add
                    )
                    nc.vector.tensor_mul(out=outt[:, rt:rt + 1], in0=st, in1=st)
            nc.sync.dma_start(out=o_sq[:, :], in_=outt[:, :])
    return _split_excess_waits(nc)


def make_in_maps(y_pred, y_true, x_values, fracture_idx):
    y_pred = np.ascontiguousarray(np.asarray(y_pred, dtype=np.float32))
    y_true = np.ascontiguousarray(np.asarray(y_true, dtype=np.float32))
    x = np.asarray(x_values, dtype=np.float32)
    idx = np.clip(np.asarray(fracture_idx).astype(np.int64), 0, N - 1)
    f = idx.astype(np.float32).reshape(B, 1)

    dx = np.diff(x)
    uniform = bool(np.all(dx == dx[0]))
    if uniform:
        scale = float(0.5 * dx[0]) ** 2 / B
    else:
        scale = 1.0 / B

    # hcl = d_f - d_0 per row (O(B) host gather; see build_nc_v3 docstring)
    rows = np.arange(B)
    d_f = y_pred[rows, idx] - y_true[rows, idx]
    d_0 = y_pred[:, 0] - y_true[:, 0]
    h = (d_f - d_0).astype(np.float32).reshape(B, 1)

    in_maps = []
    for c in range(NCORES):
        r0 = c * BS
        m = {
            "yp": y_pred[r0:r0 + BS],
            "yt": y_true[r0:r0 + BS],
            "fcl": np.ascontiguousarray(f[r0:r0 + BS]),
            "hcl": np.ascontiguousarray(h[r0:r0 + BS]),
        }
        if not uniform:
            wrow = (0.5 * dx).astype(np.float32)
            m["w"] = np.ascontiguousarray(np.broadcast_to(wrow, (P, N - 1)))
        in_maps.append(m)
    return in_maps, uniform, scale


CHUNK_K = 2048
GROUP_ROWS = 32


def kernel(y_pred, y_true, x_values, fracture_idx):
    assert y_pred.shape == (B, N), y_pred.shape
    v6 = make_in_maps_v6(y_pred, y_true, x_values, fracture_idx,
                         chunk_k=CHUNK_K, group_rows=GROUP_ROWS)
    if v6 is not None:
        in_maps, widths, full_upto, scale = v6
        key = ("v6", CHUNK_K, GROUP_ROWS, widths, full_upto)
        if key not in _nc_cache:
            _nc_cache[key] = build_nc_v6(widths, full_upto, chunk_k=CHUNK_K,
                                         group_rows=GROUP_ROWS, nqueues=2,
                                         act_full=True)
        nc = _nc_cache[key]
    else:
        in_maps, uniform, scale = make_in_maps(y_pred, y_true, x_values, fracture_idx)
        key = ("main", uniform)
        if key not in _nc_cache:
            _nc_cache[key] = (
                build_nc_v3(io_bufs=3, d_bufs=1, chunk_k=4096, alt_rings=True)
                if uniform else build_nc(uniform=False)
            )
        nc = _nc_cache[key]
    res = None
    last_err = None
    for _attempt in range(3):
        try:
            res = run_bass_kernel_spmd(nc, in_maps, list(range(NCORES)))
            break
        except Exception as e:  # sporadic NRT_EXEC_UNIT_UNRECOVERABLE on this infra
            last_err = e
            try:
                import jax
                jax.clear_backends()
            except Exception:
                pass
    if res is None:
        raise last_err
    total = 0.0
    for c in range(NCORES):
        total += np.asarray(res.results[c]["o_sq"], dtype=np.float64).sum()
    return np.asarray(total * scale, dtype=np.float32)



# revision 29
# speedup vs baseline: 1.0888x; 1.0293x over previous
"""Trainium2 Bass kernel for CustomStrainEnergyLoss (ragged fast path, v6).

Math (d = y_pred - y_true, f = clipped fracture_idx):
    pred_int_b - true_int_b = masked_trapz(d)                 (linearity)
    t_b  = sum_j 0.5*dx_j*(d_{b,j} + d_{b,j+1}) * [j < f_b]
    out  = mean_b(t_b^2)
For the uniform grid (x_values = arange), with m1 = [i<f], m2 = [i<=f]:
    sum_j (d_j + d_{j+1})*m1_j = 2*sum_i d_i*[i<f] + (d_f - d_0)
so per row the device only needs the masked sum of d over the prefix
[0, f_b); hcol = d_f - d_0 is an O(B) host gather and 0.5*dx folds into the
final host-side scalar. A general path (non-uniform dx) is kept as fallback.

Ragged exploitation (the whole speedup): elements at i >= f_b never matter,
and f is uniform in [0, N), so only ~half the 256 MiB needs to move. Host
sorts rows by f, stripes sorted rank r -> core r%8 position r//8 (cores get
identical quantile profiles, so ONE SPMD NEFF serves all 8), and the NEFF is
compile-specialized to per-group prefix widths:
  - each 128-row tile splits into groups of GR=32 sorted rows; group g loads
    [0, W_g) where W_g = roundup(max f in group) shared across cores;
  - group rows are strided across partitions (g, g+4, g+8, ...) so every
    dma_start spans all 16 SDMA engines: narrow contiguous groups run at
    ~1/16th bandwidth (measured 80 GB/s for 8-row groups vs ~430 burst /
    ~330 sustained GB/s here); fcl/hcl are host-permuted to match;
  - chunks fully inside every row's prefix ((c+1)*K <= min f of tile) skip
    the mask: yp and yt row-sums accumulate separately on the ACT engine
    (activation Copy + accum_out), subtracted at the end; only boundary
    chunks run sub + masked scalar_tensor_tensor on DVE, so compute hides
    fully under DMA;
  - rows/columns never DMA'd hold stale-but-finite data (prologue memset)
    that the (iota < f) mask multiplies to exactly 0.
Per-core traffic drops to 0.533x of full (17.06 MiB vs 32 MiB); measured
~56 us steady-state vs ~93 us for the full-read baseline at the same
methodology (slope over in-NEFF reps).

Avoided dead ends (this neuronx-cc build / hardware):
  - tensor_tensor_reduce trips "ISA wrong length" in codegen;
  - compute APs with nonzero partition start are limited to one 32-quadrant;
  - gpsimd elementwise/DMA-queue offload loses to its per-op dispatch cost;
  - >1 sync wait per instruction is rejected, so _split_excess_waits moves
    extra waits onto same-engine NoOps post-schedule.
"""

import numpy as np

from concourse import bass
import concourse.mybir as mybir
from concourse.tile import TileContext
from concourse.bass_utils import run_bass_kernel_spmd

B, N = 4096, 8192
NCORES = 8
BS = B // NCORES          # 512 rows per core
P = 128                   # partitions
RT = BS // P              # 4 row-tiles per core
K = 2048                  # column chunk
NCH = N // K              # 4 chunks

_nc_cache = {}


def _split_excess_waits(nc, maxw: int = 1):
    """Workaround for this neuronx-cc build: walrus codegen rejects any
    instruction carrying more than one sync wait ("Too many sync wait
    commands" in setupSyncWait). Move extra waits onto same-engine NoOps
    inserted immediately before the instruction (sequencer executes them in
    order, so semantics are unchanged)."""
    for b in nc.main_func.blocks:
        newlist = []
        for ins in b.instructions:
            si = ins.sync_info
            ow = list(si.on_wait) if si else []
            if len(ow) > maxw:
                extra, keep = ow[:len(ow) - maxw], ow[len(ow) - maxw:]
                for i in range(0, len(extra), maxw):
                    nop = mybir.InstNoOp(
                        name=nc.get_next_instruction_name(), ins=[], outs=[])
                    nop.engine = ins.engine
                    nop.sync_info = mybir.SyncInfo(
                        on_wait=list(extra[i:i + maxw]), on_update=[])
                    nc.register_instruction(nop)
                    newlist.append(nop)
                ins.sync_info = mybir.SyncInfo(
                    on_wait=list(keep), on_update=list(si.on_update))
            newlist.append(ins)
        b.instructions[:] = newlist
    return nc


def build_nc_v2(reps: int = 1, io_bufs: int = 3, cmp_bufs: int = 2):
    """Uniform-dx fast path.

    S_b = sum_i d_i*[i<f_b] + sum_i d_i*[i<=f_b] - d_0   (all over full rows)
    Per [128, 4096] chunk: one tensor_sub + two fused STT mask-reduces.
    2 MiB DMA loads, y_pred on the sync HWDGE ring, y_true on the scalar ring.
    """
    f32 = mybir.dt.float32
    K2 = 4096
    NCH2 = N // K2  # 2
    nc = bass.Bass()
    yp = nc.declare_dram_parameter("yp", [BS, N], f32, isOutput=False)
    yt = nc.declare_dram_parameter("yt", [BS, N], f32, isOutput=False)
    fcl = nc.declare_dram_parameter("fcl", [BS, 1], f32, isOutput=False)
    o_sq = nc.declare_dram_parameter("o_sq", [P, RT], f32, isOutput=True)

    with TileContext(nc) as tc:
        with tc.tile_pool(name="pio", bufs=io_bufs) as pio, \
             tc.tile_pool(name="pcmp", bufs=cmp_bufs) as pc, \
             tc.tile_pool(name="pq", bufs=1) as pq, \
             tc.tile_pool(name="pers", bufs=1) as pp:
            iotas = []
            for c in range(NCH2):
                it = pp.tile([P, K2], f32, tag=f"iota{c}")
                nc.gpsimd.iota(
                    it, pattern=[[1, K2]], base=c * K2, channel_multiplier=0,
                    allow_small_or_imprecise_dtypes=True,
                )
                iotas.append(it)
            outt = pp.tile([P, RT], f32, tag="outt")

            for _rep in range(reps):
                for rt in range(RT):
                    r0 = rt * P
                    fcol = pc.tile([P, 1], f32, tag="fcol")
                    nc.sync.dma_start(out=fcol, in_=fcl[r0:r0 + P, :])
                    pab = pc.tile([P, 2 * NCH2], f32, tag="pab")
                    d0 = pc.tile([P, 1], f32, tag="d0")
                    for c in range(NCH2):
                        c0 = c * K2
                        ypt = pio.tile([P, K2], f32, tag="ypt")
                        ytt = pio.tile([P, K2], f32, tag="ytt")
                        nc.sync.dma_start(out=ypt, in_=yp[r0:r0 + P, c0:c0 + K2])
                        nc.scalar.dma_start(out=ytt, in_=yt[r0:r0 + P, c0:c0 + K2])
                        d = pc.tile([P, K2], f32, tag="d")
                        nc.vector.tensor_sub(out=d, in0=ypt, in1=ytt)
                        if c == 0:
                            nc.vector.tensor_copy(out=d0, in_=d[:, 0:1])
                        q = pq.tile([P, K2], f32, tag="q")
                        nc.vector.scalar_tensor_tensor(
                            out=q, in0=iotas[c], scalar=fcol, in1=d,
                            op0=mybir.AluOpType.is_lt, op1=mybir.AluOpType.mult,
                            accum_out=pab[:, c:c + 1],
                        )
                        nc.vector.scalar_tensor_tensor(
                            out=q, in0=iotas[c], scalar=fcol, in1=d,
                            op0=mybir.AluOpType.is_le, op1=mybir.AluOpType.mult,
                            accum_out=pab[:, NCH2 + c:NCH2 + c + 1],
                        )
                    ssum = pc.tile([P, 1], f32, tag="ssum")
                    nc.vector.tensor_reduce(
                        out=ssum, in_=pab, axis=mybir.AxisListType.X, op=mybir.AluOpType.add
                    )
                    st = pc.tile([P, 1], f32, tag="st")
                    nc.vector.tensor_sub(out=st, in0=ssum, in1=d0)
                    nc.vector.tensor_mul(out=outt[:, rt:rt + 1], in0=st, in1=st)
            nc.sync.dma_start(out=o_sq[:, :], in_=outt[:, :])
    return _split_excess_waits(nc)


def build_nc_v3(reps: int = 1, io_bufs: int = 3, cmp_bufs: int = 2,
                chunk_k: int = 4096, d_bufs: int = 2, batched_fh: bool = True,
                alt_rings: bool = False):
    """Uniform-dx fast path, 2 DVE passes per element.

    Identity: with m1 = [i<f], m2 = [i<=f],  m2 - m1 = [i==f], so
        S_b = sum_i d_i*m1 + sum_i d_i*m2 - d_0 = 2*sum_i d_i*[i<f] + (d_f - d_0).
    The host supplies hcol = d_f - d_0 per row (an O(B) gather); the device
    does d = yp - yt and ONE fused mask-reduce per chunk, then
    S = 2*A + hcol, out = S^2.
    """
    f32 = mybir.dt.float32
    K2 = chunk_k
    NCH2 = N // K2
    nc = bass.Bass()
    yp = nc.declare_dram_parameter("yp", [BS, N], f32, isOutput=False)
    yt = nc.declare_dram_parameter("yt", [BS, N], f32, isOutput=False)
    fcl = nc.declare_dram_parameter("fcl", [BS, 1], f32, isOutput=False)
    hcl = nc.declare_dram_parameter("hcl", [BS, 1], f32, isOutput=False)
    o_sq = nc.declare_dram_parameter("o_sq", [P, RT], f32, isOutput=True)
    # [512,1] viewed as [128, RT]: column rt holds rows rt*128..rt*128+127
    fview = fcl.rearrange("(rt p) one -> p (rt one)", p=P)
    hview = hcl.rearrange("(rt p) one -> p (rt one)", p=P)

    with TileContext(nc) as tc:
        with tc.tile_pool(name="pio", bufs=io_bufs) as pio, \
             tc.tile_pool(name="pcmp", bufs=cmp_bufs) as pc, \
             tc.tile_pool(name="pd", bufs=d_bufs) as pd, \
             tc.tile_pool(name="pq", bufs=1) as pq, \
             tc.tile_pool(name="pers", bufs=1) as pp:
            iotas = []
            for c in range(NCH2):
                it = pp.tile([P, K2], f32, tag=f"iota{c}")
                nc.gpsimd.iota(
                    it, pattern=[[1, K2]], base=c * K2, channel_multiplier=0,
                    allow_small_or_imprecise_dtypes=True,
                )
                iotas.append(it)
            outt = pp.tile([P, RT], f32, tag="outt")

            for _rep in range(reps):
                if batched_fh:
                    fcol4 = pc.tile([P, RT], f32, tag="fcol4")
                    nc.sync.dma_start(out=fcol4, in_=fview)
                    hcol4 = pc.tile([P, RT], f32, tag="hcol4")
                    nc.sync.dma_start(out=hcol4, in_=hview)
                for rt in range(RT):
                    r0 = rt * P
                    if not batched_fh:
                        fcol4 = pc.tile([P, RT], f32, tag="fcol4")
                        nc.sync.dma_start(out=fcol4[:, rt:rt + 1], in_=fcl[r0:r0 + P, :])
                        hcol4 = pc.tile([P, RT], f32, tag="hcol4")
                        nc.sync.dma_start(out=hcol4[:, rt:rt + 1], in_=hcl[r0:r0 + P, :])
                    pab = pc.tile([P, NCH2], f32, tag="pab")
                    for c in range(NCH2):
                        c0 = c * K2
                        ypt = pio.tile([P, K2], f32, tag="ypt")
                        ytt = pio.tile([P, K2], f32, tag="ytt")
                        e0, e1 = (nc.sync, nc.scalar)
                        if alt_rings and (rt * NCH2 + c) % 2 == 1:
                            e0, e1 = (nc.scalar, nc.sync)
                        e0.dma_start(out=ypt, in_=yp[r0:r0 + P, c0:c0 + K2])
                        e1.dma_start(out=ytt, in_=yt[r0:r0 + P, c0:c0 + K2])
                        d = pd.tile([P, K2], f32, tag="d")
                        nc.vector.tensor_sub(out=d, in0=ypt, in1=ytt)
                        q = pq.tile([P, K2], f32, tag="q")
                        nc.vector.scalar_tensor_tensor(
                            out=q, in0=iotas[c], scalar=fcol4[:, rt:rt + 1], in1=d,
                            op0=mybir.AluOpType.is_lt, op1=mybir.AluOpType.mult,
                            accum_out=pab[:, c:c + 1],
                        )
                    ssum = pc.tile([P, 1], f32, tag="ssum")
                    if NCH2 > 1:
                        nc.vector.tensor_reduce(
                            out=ssum, in_=pab, axis=mybir.AxisListType.X,
                            op=mybir.AluOpType.add,
                        )
                    else:
                        ssum = pab
                    st = pc.tile([P, 1], f32, tag="st")
                    nc.vector.scalar_tensor_tensor(
                        out=st, in0=ssum, scalar=2.0, in1=hcol4[:, rt:rt + 1],
                        op0=mybir.AluOpType.mult, op1=mybir.AluOpType.add,
                    )
                    nc.vector.tensor_mul(out=outt[:, rt:rt + 1], in0=st, in1=st)
            nc.sync.dma_start(out=o_sq[:, :], in_=outt[:, :])
    return _split_excess_waits(nc)


def build_nc_v4(bounds, chunk_k: int = 512, reps: int = 1, io_bufs: int = 4,
                cmp_bufs: int = 2, d_bufs: int = 2, full_upto=None,
                do_dma: bool = True, do_compute: bool = True):
    """Ragged fast path: rows are pre-sorted by fracture_idx on host, so for
    column chunk c only the partition-suffix of rows with f > c*K contributes;
    everything below the suffix is masked to exactly 0 anyway. We skip both
    the DMA and the vector work for those rows: traffic drops from B*N to
    ~sum(f) elements (~2x for uniform f). `bounds[rt][c]` is the first
    partition that needs chunk c in row-tile rt (shared across cores = min
    over cores, so one SPMD NEFF serves all 8).
    Per-tile accumulator pab is memset to 0, so skipped (row, chunk) cells
    contribute nothing; loaded-but-masked rows contribute (iota<f)*d = 0.
    """
    f32 = mybir.dt.float32
    K2 = chunk_k
    NCH2 = N // K2
    nc = bass.Bass()
    yp = nc.declare_dram_parameter("yp", [BS, N], f32, isOutput=False)
    yt = nc.declare_dram_parameter("yt", [BS, N], f32, isOutput=False)
    fcl = nc.declare_dram_parameter("fcl", [BS, 1], f32, isOutput=False)
    hcl = nc.declare_dram_parameter("hcl", [BS, 1], f32, isOutput=False)
    o_sq = nc.declare_dram_parameter("o_sq", [P, RT], f32, isOutput=True)
    fview = fcl.rearrange("(rt p) one -> p (rt one)", p=P)
    hview = hcl.rearrange("(rt p) one -> p (rt one)", p=P)

    with TileContext(nc) as tc:
        with tc.tile_pool(name="pio", bufs=io_bufs) as pio, \
             tc.tile_pool(name="pcmp", bufs=cmp_bufs) as pc, \
             tc.tile_pool(name="pd", bufs=d_bufs) as pd, \
             tc.tile_pool(name="pq", bufs=1) as pq, \
             tc.tile_pool(name="pers", bufs=1) as pp:
            iota = pp.tile([P, N], f32, tag="iota")
            nc.gpsimd.iota(
                iota, pattern=[[1, N]], base=0, channel_multiplier=0,
                allow_small_or_imprecise_dtypes=True,
            )
            outt = pp.tile([P, RT], f32, tag="outt")

            # Compute APs must start on a 32-partition quadrant boundary, so
            # vector ops below run on [ps32:P) while DMA fills only [ps:P).
            # Rows in [ps32, ps) are masked to 0 by (iota < f); memset every
            # io buffer once so their stale contents are finite (0*x == 0).
            for _i in range(io_bufs):
                zyp = pio.tile([P, K2], f32, tag="ypt")
                nc.vector.memset(zyp, 0.0)
                zyt = pio.tile([P, K2], f32, tag="ytt")
                nc.vector.memset(zyt, 0.0)

            flip = 0
            for _rep in range(reps):
                fcol4 = pc.tile([P, RT], f32, tag="fcol4")
                nc.sync.dma_start(out=fcol4, in_=fview)
                hcol4 = pc.tile([P, RT], f32, tag="hcol4")
                nc.sync.dma_start(out=hcol4, in_=hview)
                for rt in range(RT):
                    r0 = rt * P
                    pab = pc.tile([P, NCH2], f32, tag="pab")
                    nc.vector.memset(pab, 0.0)
                    for c in range(NCH2):
                        ps = bounds[rt][c]
                        if ps >= P:
                            break  # f sorted -> later chunks needed by nobody
                        c0 = c * K2
                        ypt = pio.tile([P, K2], f32, tag="ypt")
                        ytt = pio.tile([P, K2], f32, tag="ytt")
                        e0, e1 = (nc.sync, nc.scalar) if flip == 0 else (nc.scalar, nc.sync)
                        flip ^= 1
                        if do_dma:
                            e0.dma_start(out=ypt[ps:P, :], in_=yp[r0 + ps:r0 + P, c0:c0 + K2])
                            e1.dma_start(out=ytt[ps:P, :], in_=yt[r0 + ps:r0 + P, c0:c0 + K2])
                        if not do_compute:
                            continue
                        # Compute on all 128 partitions (nonzero partition
                        # starts are limited to one 32-quadrant); rows below
                        # ps hold stale-but-finite data and mask to 0.
                        d = pd.tile([P, K2], f32, tag="d")
                        if full_upto is not None and c < full_upto[rt]:
                            # Chunk lies inside every row's prefix: fuse
                            # subtract + row-sum in one DVE pass, no mask.
                            nc.vector.tensor_tensor_reduce(
                                out=d, in0=ypt, in1=ytt, scale=1.0, scalar=0.0,
                                op0=mybir.AluOpType.subtract,
                                op1=mybir.AluOpType.add,
                                accum_out=pab[:, c:c + 1],
                            )
                            continue
                        nc.vector.tensor_sub(out=d, in0=ypt, in1=ytt)
                        q = pq.tile([P, K2], f32, tag="q")
                        nc.vector.scalar_tensor_tensor(
                            out=q, in0=iota[:, c0:c0 + K2],
                            scalar=fcol4[:, rt:rt + 1], in1=d,
                            op0=mybir.AluOpType.is_lt, op1=mybir.AluOpType.mult,
                            accum_out=pab[:, c:c + 1],
                        )
                    ssum = pc.tile([P, 1], f32, tag="ssum")
                    nc.vector.tensor_reduce(
                        out=ssum, in_=pab, axis=mybir.AxisListType.X,
                        op=mybir.AluOpType.add,
                    )
                    st = pc.tile([P, 1], f32, tag="st")
                    nc.vector.scalar_tensor_tensor(
                        out=st, in0=ssum, scalar=2.0, in1=hcol4[:, rt:rt + 1],
                        op0=mybir.AluOpType.mult, op1=mybir.AluOpType.add,
                    )
                    nc.vector.tensor_mul(out=outt[:, rt:rt + 1], in0=st, in1=st)
            nc.sync.dma_start(out=o_sq[:, :], in_=outt[:, :])
    return _split_excess_waits(nc)


def build_nc_v5(widths, full_upto, chunk_k: int = 2048, reps: int = 1,
                io_bufs: int = 2, cmp_bufs: int = 2, d_bufs: int = 1,
                group_rows: int = 8, do_dma: bool = True,
                do_compute: bool = True):
    """Ragged fast path, row-group full-prefix DMA.

    v4's [row-suffix x 512-col] DMAs made 2 KiB descriptors and died on
    per-descriptor overhead (250 GB/s dma-only). Here each DMA loads
    [group_rows x W_g] where W_g = the group's shared prefix width, so every
    descriptor is one FULL row prefix (up to 32 KiB). widths[rt][g] (elems,
    nondecreasing in g) and full_upto[rt] (chunks fully inside every row's
    prefix) are host-computed from sorted f, shared across cores (max over
    cores) so one SPMD NEFF serves all 8.
    Full chunks need no mask: row-sums of yp and yt accumulate separately
    (single-input tensor_reduce; tensor_tensor_reduce trips "ISA wrong
    length" in this neuronx-cc build) and subtract at the end. Boundary
    chunks use sub + masked STT. Rows keep real data on [0, W_g); columns
    beyond W_g hold stale-but-finite values (prologue memset) that the
    (iota < f) mask zeroes.
    """
    f32 = mybir.dt.float32
    K2 = chunk_k
    GR = group_rows
    NG = P // GR
    nc = bass.Bass()
    yp = nc.declare_dram_parameter("yp", [BS, N], f32, isOutput=False)
    yt = nc.declare_dram_parameter("yt", [BS, N], f32, isOutput=False)
    fcl = nc.declare_dram_parameter("fcl", [BS, 1], f32, isOutput=False)
    hcl = nc.declare_dram_parameter("hcl", [BS, 1], f32, isOutput=False)
    o_sq = nc.declare_dram_parameter("o_sq", [P, RT], f32, isOutput=True)
    fview = fcl.rearrange("(rt p) one -> p (rt one)", p=P)
    hview = hcl.rearrange("(rt p) one -> p (rt one)", p=P)

    wmax = [max(widths[rt]) for rt in range(RT)]

    with TileContext(nc) as tc:
        with tc.tile_pool(name="pio", bufs=io_bufs) as pio, \
             tc.tile_pool(name="pcmp", bufs=cmp_bufs) as pc, \
             tc.tile_pool(name="pd", bufs=d_bufs) as pd, \
             tc.tile_pool(name="pq", bufs=1) as pq, \
             tc.tile_pool(name="pers", bufs=1) as pp:
            iota = pp.tile([P, N], f32, tag="iota")
            nc.gpsimd.iota(
                iota, pattern=[[1, N]], base=0, channel_multiplier=0,
                allow_small_or_imprecise_dtypes=True,
            )
            outt = pp.tile([P, RT], f32, tag="outt")
            nc.vector.memset(outt, 0.0)

            for _i in range(io_bufs):
                zyp = pio.tile([P, N], f32, tag="ypt")
                nc.vector.memset(zyp, 0.0)
                zyt = pio.tile([P, N], f32, tag="ytt")
                nc.vector.memset(zyt, 0.0)

            flip = 0
            for _rep in range(reps):
                fcol4 = pc.tile([P, RT], f32, tag="fcol4")
                nc.sync.dma_start(out=fcol4, in_=fview)
                hcol4 = pc.tile([P, RT], f32, tag="hcol4")
                nc.sync.dma_start(out=hcol4, in_=hview)
                for rt in range(RT):
                    r0 = rt * P
                    wt = wmax[rt]
                    nch_t = (wt + K2 - 1) // K2
                    ypt = pio.tile([P, N], f32, tag="ypt")
                    ytt = pio.tile([P, N], f32, tag="ytt")
                    if do_dma:
                        for g in range(NG):
                            wg = widths[rt][g]
                            if wg <= 0:
                                continue
                            p0 = g * GR
                            e0, e1 = (nc.sync, nc.scalar) if flip == 0 else (nc.scalar, nc.sync)
                            flip ^= 1
                            e0.dma_start(out=ypt[p0:p0 + GR, :wg],
                                         in_=yp[r0 + p0:r0 + p0 + GR, :wg])
                            e1.dma_start(out=ytt[p0:p0 + GR, :wg],
                                         in_=yt[r0 + p0:r0 + p0 + GR, :wg])
                    if not do_compute:
                        continue
                    # pabp accumulates +sum(yp)/-masked contributions per
                    # chunk column; pabt the matching yt sums (full chunks
                    # only; boundary chunks fold the whole (yp-yt) masked
                    # sum into pabp and leave pabt's column at 0).
                    pabp = pc.tile([P, N // K2], f32, tag="pabp")
                    nc.vector.memset(pabp, 0.0)
                    pabt = pc.tile([P, N // K2], f32, tag="pabt")
                    nc.vector.memset(pabt, 0.0)
                    for c in range(nch_t):
                        c0 = c * K2
                        w = min(K2, wt - c0)
                        if c < full_upto[rt]:
                            nc.vector.tensor_reduce(
                                out=pabp[:, c:c + 1], in_=ypt[:, c0:c0 + w],
                                axis=mybir.AxisListType.X, op=mybir.AluOpType.add,
                            )
                            nc.vector.tensor_reduce(
                                out=pabt[:, c:c + 1], in_=ytt[:, c0:c0 + w],
                                axis=mybir.AxisListType.X, op=mybir.AluOpType.add,
                            )
                            continue
                        d = pd.tile([P, K2], f32, tag="d")
                        nc.vector.tensor_sub(out=d[:, :w], in0=ypt[:, c0:c0 + w],
                                             in1=ytt[:, c0:c0 + w])
                        q = pq.tile([P, K2], f32, tag="q")
                        nc.vector.scalar_tensor_tensor(
                            out=q[:, :w], in0=iota[:, c0:c0 + w],
                            scalar=fcol4[:, rt:rt + 1], in1=d[:, :w],
                            op0=mybir.AluOpType.is_lt, op1=mybir.AluOpType.mult,
                            accum_out=pabp[:, c:c + 1],
                        )
                    ssp = pc.tile([P, 1], f32, tag="ssp")
                    nc.vector.tensor_reduce(
                        out=ssp, in_=pabp, axis=mybir.AxisListType.X,
                        op=mybir.AluOpType.add,
                    )
                    sst = pc.tile([P, 1], f32, tag="sst")
                    nc.vector.tensor_reduce(
                        out=sst, in_=pabt, axis=mybir.AxisListType.X,
                        op=mybir.AluOpType.add,
                    )
                    sdf = pc.tile([P, 1], f32, tag="sdf")
                    nc.vector.tensor_sub(out=sdf, in0=ssp, in1=sst)
                    st = pc.tile([P, 1], f32, tag="st")
                    nc.vector.scalar_tensor_tensor(
                        out=st, in0=sdf, scalar=2.0, in1=hcol4[:, rt:rt + 1],
                        op0=mybir.AluOpType.mult, op1=mybir.AluOpType.add,
                    )
                    nc.vector.tensor_mul(out=outt[:, rt:rt + 1], in0=st, in1=st)
            nc.sync.dma_start(out=o_sq[:, :], in_=outt[:, :])
    return _split_excess_waits(nc)


def make_in_maps_v5(y_pred, y_true, x_values, fracture_idx, chunk_k: int = 2048,
                    group_rows: int = 8, align: int = 128):
    """Sharding + group widths for v5. Returns None when dx non-uniform."""
    x = np.asarray(x_values, dtype=np.float32)
    dx = np.diff(x)
    if not (dx.size > 0 and bool(np.all(dx == dx[0]))):
        return None
    scale = float(0.5 * dx[0]) ** 2 / B

    y_pred = np.ascontiguousarray(np.asarray(y_pred, dtype=np.float32))
    y_true = np.ascontiguousarray(np.asarray(y_true, dtype=np.float32))
    idx = np.clip(np.asarray(fracture_idx).astype(np.int64), 0, N - 1)
    rows_all = np.arange(B)
    d_f = y_pred[rows_all, idx] - y_true[rows_all, idx]
    d_0 = y_pred[:, 0] - y_true[:, 0]
    h = (d_f - d_0).astype(np.float32)
    f = idx.astype(np.float32)

    order = np.argsort(idx, kind="stable")
    in_maps = []
    fmat = np.empty((NCORES, BS), np.int64)
    for m in range(NCORES):
        rows = order[m::NCORES]
        fmat[m] = idx[rows]
        in_maps.append({
            "yp": np.ascontiguousarray(y_pred[rows]),
            "yt": np.ascontiguousarray(y_true[rows]),
            "fcl": np.ascontiguousarray(f[rows].reshape(BS, 1)),
            "hcl": np.ascontiguousarray(h[rows].reshape(BS, 1)),
        })
    GR = group_rows
    NG = P // GR
    widths = []
    full_upto = []
    for rt in range(RT):
        row_w = []
        for g in range(NG):
            lo = rt * P + g * GR
            w = int(fmat[:, lo:lo + GR].max())  # max f over group, all cores
            w = min(N, -(-w // align) * align)
            row_w.append(w)
        widths.append(tuple(row_w))
        minf = int(fmat[:, rt * P].min())
        full_upto.append(minf // chunk_k)
    return in_maps, tuple(widths), tuple(full_upto), scale


def build_nc_v6(widths, full_upto, chunk_k: int = 2048, reps: int = 1,
                io_bufs: int = 2, cmp_bufs: int = 2, d_bufs: int = 1,
                group_rows: int = 16, nqueues: int = 2,
                do_dma: bool = True, do_compute: bool = True,
                act_full: bool = False, gp_sub: bool = False,
                fh_gp: bool = False, fuse_io: bool = False):
    """Ragged fast path, engine-spread row-group DMA.

    Per-dma_start throughput is (partition span / 8) x ~27 GB/s SDMA engines
    (measured: 8-row groups 80 GB/s, 32-row 232 GB/s, 128-row ~360 GB/s), so
    v6 strides each group's GR rows across partitions g, g+NG, g+2*NG, ...
    (NG = 128/GR): every group DMA then touches all 16 SDMA engines at full
    rate while keeping the fine per-group prefix widths (~2% overshoot).
    DRAM stays row-sorted and contiguous; fcl/hcl are host-permuted to
    partition order, so compute is identical to v5.
    """
    f32 = mybir.dt.float32
    K2 = chunk_k
    GR = group_rows
    NG = P // GR
    nc = bass.Bass()
    if fuse_io:
        yy = nc.declare_dram_parameter("yy", [BS, 2 * N], f32, isOutput=False)
        yy3 = yy.rearrange("b (t n) -> b t n", t=2)
    else:
        yp = nc.declare_dram_parameter("yp", [BS, N], f32, isOutput=False)
        yt = nc.declare_dram_parameter("yt", [BS, N], f32, isOutput=False)
    fcl = nc.declare_dram_parameter("fcl", [BS, 1], f32, isOutput=False)
    hcl = nc.declare_dram_parameter("hcl", [BS, 1], f32, isOutput=False)
    o_sq = nc.declare_dram_parameter("o_sq", [P, RT], f32, isOutput=True)
    fview = fcl.rearrange("(rt p) one -> p (rt one)", p=P)
    hview = hcl.rearrange("(rt p) one -> p (rt one)", p=P)

    wmax = [max(widths[rt]) for rt in range(RT)]
    queues = [nc.sync, nc.scalar, nc.gpsimd][:nqueues]

    with TileContext(nc) as tc:
        with tc.tile_pool(name="pio", bufs=io_bufs) as pio, \
             tc.tile_pool(name="pcmp", bufs=cmp_bufs) as pc, \
             tc.tile_pool(name="pd", bufs=d_bufs) as pd, \
             tc.tile_pool(name="pq", bufs=1) as pq, \
             tc.tile_pool(name="pers", bufs=1) as pp:
            iota = pp.tile([P, N], f32, tag="iota")
            nc.gpsimd.iota(
                iota, pattern=[[1, N]], base=0, channel_multiplier=0,
                allow_small_or_imprecise_dtypes=True,
            )
            outt = pp.tile([P, RT], f32, tag="outt")
            nc.vector.memset(outt, 0.0)

            if fuse_io:
                for _i in range(io_bufs):
                    zyy = pio.tile([P, 2 * N], f32, tag="yyt")
                    nc.vector.memset(zyy, 0.0)
            else:
                for _i in range(io_bufs):
                    zyp = pio.tile([P, N], f32, tag="ypt")
                    nc.vector.memset(zyp, 0.0)
                    zyt = pio.tile([P, N], f32, tag="ytt")
                    nc.vector.memset(zyt, 0.0)

            qi = 0
            for _rep in range(reps):
                fq = nc.gpsimd if fh_gp else nc.sync
                fcol4 = pc.tile([P, RT], f32, tag="fcol4")
                fq.dma_start(out=fcol4, in_=fview)
                hcol4 = pc.tile([P, RT], f32, tag="hcol4")
                fq.dma_start(out=hcol4, in_=hview)
                for rt in range(RT):
                    r0 = rt * P
                    wt = wmax[rt]
                    nch_t = (wt + K2 - 1) // K2
                    if fuse_io:
                        yyt = pio.tile([P, 2 * N], f32, tag="yyt")
                        yyt3 = yyt.rearrange("p (t n) -> p t n", t=2)
                        ypt = yyt[:, 0:N]
                        ytt = yyt[:, N:2 * N]
                        if do_dma:
                            for g in range(NG):
                                wg = widths[rt][g]
                                if wg <= 0:
                                    continue
                                s0 = r0 + GR * g
                                queues[qi % nqueues].dma_start(
                                    out=yyt3[g::NG, :, :wg],
                                    in_=yy3[s0:s0 + GR, :, :wg])
                                qi += 1
                    else:
                        ypt = pio.tile([P, N], f32, tag="ypt")
                        ytt = pio.tile([P, N], f32, tag="ytt")
                        if do_dma:
                            for g in range(NG):
                                wg = widths[rt][g]
                                if wg <= 0:
                                    continue
                                s0 = r0 + GR * g
                                queues[qi % nqueues].dma_start(
                                    out=ypt[g::NG, :wg], in_=yp[s0:s0 + GR, :wg])
                                queues[(qi + 1) % nqueues].dma_start(
                                    out=ytt[g::NG, :wg], in_=yt[s0:s0 + GR, :wg])
                                qi += 2
                    if not do_compute:
                        continue
                    pabp = pc.tile([P, N // K2], f32, tag="pabp")
                    nc.vector.memset(pabp, 0.0)
                    pabt = pc.tile([P, N // K2], f32, tag="pabt")
                    nc.vector.memset(pabt, 0.0)
                    for c in range(nch_t):
                        c0 = c * K2
                        w = min(K2, wt - c0)
                        if c < full_upto[rt]:
                            if act_full:
                                ascr = pq.tile([P, K2], f32, tag="ascr")
                                nc.scalar.activation(
                                    out=ascr[:, :w], in_=ypt[:, c0:c0 + w],
                                    func=mybir.ActivationFunctionType.Copy,
                                    accum_out=pabp[:, c:c + 1],
                                )
                                nc.scalar.activation(
                                    out=ascr[:, :w], in_=ytt[:, c0:c0 + w],
                                    func=mybir.ActivationFunctionType.Copy,
                                    accum_out=pabt[:, c:c + 1],
                                )
                                continue
                            nc.vector.tensor_reduce(
                                out=pabp[:, c:c + 1], in_=ypt[:, c0:c0 + w],
                                axis=mybir.AxisListType.X, op=mybir.AluOpType.add,
                            )
                            nc.vector.tensor_reduce(
                                out=pabt[:, c:c + 1], in_=ytt[:, c0:c0 + w],
                                axis=mybir.AxisListType.X, op=mybir.AluOpType.add,
                            )
                            continue
                        d = pd.tile([P, K2], f32, tag="d")
                        sub_eng = nc.gpsimd if gp_sub else nc.vector
                        sub_eng.tensor_sub(out=d[:, :w], in0=ypt[:, c0:c0 + w],
                                           in1=ytt[:, c0:c0 + w])
                        q = pq.tile([P, K2], f32, tag="q")
                        nc.vector.scalar_tensor_tensor(
                            out=q[:, :w], in0=iota[:, c0:c0 + w],
                            scalar=fcol4[:, rt:rt + 1], in1=d[:, :w],
                            op0=mybir.AluOpType.is_lt, op1=mybir.AluOpType.mult,
                            accum_out=pabp[:, c:c + 1],
                        )
                    ssp = pc.tile([P, 1], f32, tag="ssp")
                    nc.vector.tensor_reduce(
                        out=ssp, in_=pabp, axis=mybir.AxisListType.X,
                        op=mybir.AluOpType.add,
                    )
                    sst = pc.tile([P, 1], f32, tag="sst")
                    nc.vector.tensor_reduce(
                        out=sst, in_=pabt, axis=mybir.AxisListType.X,
                        op=mybir.AluOpType.add,
                    )
                    sdf = pc.tile([P, 1], f32, tag="sdf")
                    nc.vector.tensor_sub(out=sdf, in0=ssp, in1=sst)
                    st = pc.tile([P, 1], f32, tag="st")
                    nc.vector.scalar_tensor_tensor(
                        out=st, in0=sdf, scalar=2.0, in1=hcol4[:, rt:rt + 1],
                        op0=mybir.AluOpType.mult, op1=mybir.AluOpType.add,
                    )
                    nc.vector.tensor_mul(out=outt[:, rt:rt + 1], in0=st, in1=st)
            nc.sync.dma_start(out=o_sq[:, :], in_=outt[:, :])
    return _split_excess_waits(nc)


def make_in_maps_v6(y_pred, y_true, x_values, fracture_idx, chunk_k: int = 2048,
                    group_rows: int = 16, align: int = 128, fuse_io: bool = False):
    """Sharding + group widths for v6 (strided-partition groups).

    yp/yt stay in fully sorted row order (DMA sources are contiguous row
    blocks). fcl/hcl are permuted to PARTITION order: partition p of tile rt
    holds sorted row j(p) = GR*(p % NG) + p // NG, matching the strided DMA
    dest ypt[g::NG] for group g's rows [GR*g, GR*(g+1)).
    """
    x = np.asarray(x_values, dtype=np.float32)
    dx = np.diff(x)
    if not (dx.size > 0 and bool(np.all(dx == dx[0]))):
        return None
    scale = float(0.5 * dx[0]) ** 2 / B

    y_pred = np.ascontiguousarray(np.asarray(y_pred, dtype=np.float32))
    y_true = np.ascontiguousarray(np.asarray(y_true, dtype=np.float32))
    idx = np.clip(np.asarray(fracture_idx).astype(np.int64), 0, N - 1)
    rows_all = np.arange(B)
    d_f = y_pred[rows_all, idx] - y_true[rows_all, idx]
    d_0 = y_pred[:, 0] - y_true[:, 0]
    h = (d_f - d_0).astype(np.float32)
    f = idx.astype(np.float32)

    GR = group_rows
    NG = P // GR
    # partition p <- sorted row j(p) within each 128-row tile
    p_arr = np.arange(P)
    j_of_p = GR * (p_arr % NG) + p_arr // NG
    perm_tile = np.concatenate([rt * P + j_of_p for rt in range(RT)])

    order = np.argsort(idx, kind="stable")
    in_maps = []
    fmat = np.empty((NCORES, BS), np.int64)
    for m in range(NCORES):
        rows = order[m::NCORES]
        fmat[m] = idx[rows]
        rows_p = rows[perm_tile]
        im = {
            "fcl": np.ascontiguousarray(f[rows_p].reshape(BS, 1)),
            "hcl": np.ascontiguousarray(h[rows_p].reshape(BS, 1)),
        }
        if fuse_io:
            yy = np.empty((BS, 2, N), np.float32)
            yy[:, 0, :] = y_pred[rows]
            yy[:, 1, :] = y_true[rows]
            im["yy"] = yy.reshape(BS, 2 * N)
        else:
            im["yp"] = np.ascontiguousarray(y_pred[rows])
            im["yt"] = np.ascontiguousarray(y_true[rows])
        in_maps.append(im)
    widths = []
    full_upto = []
    for rt in range(RT):
        row_w = []
        for g in range(NG):
            lo = rt * P + g * GR
            w = int(fmat[:, lo:lo + GR].max())
            w = min(N, -(-w // align) * align)
            row_w.append(w)
        widths.append(tuple(row_w))
        minf = int(fmat[:, rt * P].min())
        full_upto.append(minf // chunk_k)
    return in_maps, tuple(widths), tuple(full_upto), scale


def make_in_maps_v4(y_pred, y_true, x_values, fracture_idx, chunk_k: int = 512):
    """Sorted+striped data-parallel sharding for the ragged fast path.

    Returns (in_maps, bounds, scale) or None when dx is not uniform (caller
    falls back to the general path). Global sort rank r -> core r%8 position
    r//8, so every core sees the same f quantiles and one set of suffix
    bounds (min over cores) serves all cores' SPMD NEFF.
    """
    x = np.asarray(x_values, dtype=np.float32)
    dx = np.diff(x)
    if not (dx.size > 0 and bool(np.all(dx == dx[0]))):
        return None
    scale = float(0.5 * dx[0]) ** 2 / B

    y_pred = np.ascontiguousarray(np.asarray(y_pred, dtype=np.float32))
    y_true = np.ascontiguousarray(np.asarray(y_true, dtype=np.float32))
    idx = np.clip(np.asarray(fracture_idx).astype(np.int64), 0, N - 1)
    rows_all = np.arange(B)
    d_f = y_pred[rows_all, idx] - y_true[rows_all, idx]
    d_0 = y_pred[:, 0] - y_true[:, 0]
    h = (d_f - d_0).astype(np.float32)
    f = idx.astype(np.float32)

    order = np.argsort(idx, kind="stable")
    K2 = chunk_k
    NCH2 = N // K2
    in_maps = []
    fmat = np.empty((NCORES, BS), np.int64)
    for m in range(NCORES):
        rows = order[m::NCORES]
        fmat[m] = idx[rows]
        in_maps.append({
            "yp": np.ascontiguousarray(y_pred[rows]),
            "yt": np.ascontiguousarray(y_true[rows]),
            "fcl": np.ascontiguousarray(f[rows].reshape(BS, 1)),
            "hcl": np.ascontiguousarray(h[rows].reshape(BS, 1)),
        })
    bounds = []
    full_upto = []
    for rt in range(RT):
        row_b = []
        for c in range(NCH2):
            ps = P
            for m in range(NCORES):
                ft = fmat[m, rt * P:(rt + 1) * P]
                ps = min(ps, int(np.searchsorted(ft, c * K2, side="right")))
            row_b.append(ps)
        bounds.append(tuple(row_b))
        minf = int(fmat[:, rt * P].min())  # rows ascending -> tile min f
        full_upto.append(minf // K2)
    return in_maps, tuple(bounds), tuple(full_upto), scale


def build_nc(uniform: bool = True, reps: int = 1, io_bufs: int = 3, cmp_bufs: int = 2):
    f32 = mybir.dt.float32
    nc = bass.Bass()
    yp = nc.declare_dram_parameter("yp", [BS, N], f32, isOutput=False)
    yt = nc.declare_dram_parameter("yt", [BS, N], f32, isOutput=False)
    fcl = nc.declare_dram_parameter("fcl", [BS, 1], f32, isOutput=False)
    w = None
    if not uniform:
        w = nc.declare_dram_parameter("w", [P, N - 1], f32, isOutput=False)
    o_sq = nc.declare_dram_parameter("o_sq", [P, RT], f32, isOutput=True)

    with TileContext(nc) as tc:
        with tc.tile_pool(name="pio", bufs=io_bufs) as pio, \
             tc.tile_pool(name="pcmp", bufs=cmp_bufs) as pc, \
             tc.tile_pool(name="pers", bufs=1) as pp:
            # One-time: per-chunk f32 iota rows (values are exact ints < 2^24).
            iotas = []
            wts = []
            for c in range(NCH):
                seg = K if c < NCH - 1 else K - 1
                it = pp.tile([P, seg], f32, tag=f"iota{c}")
                nc.gpsimd.iota(
                    it, pattern=[[1, seg]], base=c * K, channel_multiplier=0,
                    allow_small_or_imprecise_dtypes=True,
                )
                iotas.append(it)
                if not uniform:
                    wt = pp.tile([P, seg], f32, tag=f"w{c}")
                    nc.sync.dma_start(out=wt, in_=w[:, c * K:c * K + seg])
                    wts.append(wt)
            outt = pp.tile([P, RT], f32, tag="outt")

            for _rep in range(reps):
                for rt in range(RT):
                    r0 = rt * P
                    fcol = pc.tile([P, 1], f32, tag="fcol")
                    nc.sync.dma_start(out=fcol, in_=fcl[r0:r0 + P, :])
                    p4 = pc.tile([P, NCH], f32, tag="p4")
                    for c in range(NCH):
                        lw = K + 1 if c < NCH - 1 else K   # load width
                        seg = lw - 1                       # segments
                        c0 = c * K
                        ypt = pio.tile([P, K + 1], f32, tag="ypt")
                        ytt = pio.tile([P, K + 1], f32, tag="ytt")
                        nc.sync.dma_start(out=ypt[:, :lw], in_=yp[r0:r0 + P, c0:c0 + lw])
                        nc.sync.dma_start(out=ytt[:, :lw], in_=yt[r0:r0 + P, c0:c0 + lw])
                        d = pc.tile([P, K + 1], f32, tag="d")
                        nc.vector.tensor_sub(out=d[:, :lw], in0=ypt[:, :lw], in1=ytt[:, :lw])
                        s = pc.tile([P, K], f32, tag="s")
                        nc.vector.tensor_add(out=s[:, :seg], in0=d[:, 0:seg], in1=d[:, 1:seg + 1])
                        src = s
                        if not uniform:
                            u = pc.tile([P, K], f32, tag="u")
                            nc.vector.tensor_mul(out=u[:, :seg], in0=s[:, :seg], in1=wts[c][:, :seg])
                            src = u
                        q = pc.tile([P, K], f32, tag="q")
                        nc.vector.scalar_tensor_tensor(
                            out=q[:, :seg], in0=iotas[c][:, :seg], scalar=fcol,
                            in1=src[:, :seg],
                            op0=mybir.AluOpType.is_lt, op1=mybir.AluOpType.mult,
                            accum_out=p4[:, c:c + 1],
                        )
                    st = pc.tile([P, 1], f32, tag="st")
                    nc.vector.tensor_reduce(
                        out=st, in_=p4, axis=mybir.AxisListType.X, op=mybir.AluOpType.add
                    )
                    nc.vector.tensor_mul(out=outt[:, rt:rt + 1], in0=st, in1=st)
            nc.sync.dma_start(out=o_sq[:, :], in_=outt[:, :])
    return _split_excess_waits(nc)


def make_in_maps(y_pred, y_true, x_values, fracture_idx):
    y_pred = np.ascontiguousarray(np.asarray(y_pred, dtype=np.float32))
    y_true = np.ascontiguousarray(np.asarray(y_true, dtype=np.float32))
    x = np.asarray(x_values, dtype=np.float32)
    idx = np.clip(np.asarray(fracture_idx).astype(np.int64), 0, N - 1)
    f = idx.astype(np.float32).reshape(B, 1)

    dx = np.diff(x)
    uniform = bool(np.all(dx == dx[0]))
    if uniform:
        scale = float(0.5 * dx[0]) ** 2 / B
    else:
        scale = 1.0 / B

    # hcl = d_f - d_0 per row (O(B) host gather; see build_nc_v3 docstring)
    rows = np.arange(B)
    d_f = y_pred[rows, idx] - y_true[rows, idx]
    d_0 = y_pred[:, 0] - y_true[:, 0]
    h = (d_f - d_0).astype(np.float32).reshape(B, 1)

    in_maps = []
    for c in range(NCORES):
        r0 = c * BS
        m = {
            "yp": y_pred[r0:r0 + BS],
            "yt": y_true[r0:r0 + BS],
            "fcl": np.ascontiguousarray(f[r0:r0 + BS]),
            "hcl": np.ascontiguousarray(h[r0:r0 + BS]),
        }
        if not uniform:
            wrow = (0.5 * dx).astype(np.float32)
            m["w"] = np.ascontiguousarray(np.broadcast_to(wrow, (P, N - 1)))
        in_maps.append(m)
    return in_maps, uniform, scale


CHUNK_K = 2048
GROUP_ROWS = 32


def kernel(y_pred, y_true, x_values, fracture_idx):
    assert y_pred.shape == (B, N), y_pred.shape
    v6 = make_in_maps_v6(y_pred, y_true, x_values, fracture_idx,
                         chunk_k=CHUNK_K, group_rows=GROUP_ROWS)
    if v6 is not None:
        in_maps, widths, full_upto, scale = v6
        key = ("v6", CHUNK_K, GROUP_ROWS, widths, full_upto)
        if key not in _nc_cache:
            _nc_cache[key] = build_nc_v6(widths, full_upto, chunk_k=CHUNK_K,
                                         group_rows=GROUP_ROWS, nqueues=2,
                                         act_full=True)
        nc = _nc_cache[key]
    else:
        in_maps, uniform, scale = make_in_maps(y_pred, y_true, x_values, fracture_idx)
        key = ("main", uniform)
        if key not in _nc_cache:
            _nc_cache[key] = (
                build_nc_v3(io_bufs=3, d_bufs=1, chunk_k=4096, alt_rings=True)
                if uniform else build_nc(uniform=False)
            )
        nc = _nc_cache[key]
    res = None
    last_err = None
    for _attempt in range(3):
        try:
            res = run_bass_kernel_spmd(nc, in_maps, list(range(NCORES)))
            break
        except Exception as e:  # sporadic NRT_EXEC_UNIT_UNRECOVERABLE on this infra
            last_err = e
            try:
                import jax
                jax.clear_backends()
            except Exception:
                pass
    if res is None:
        raise last_err
    total = 0.0
    for c in range(NCORES):
        total += np.asarray(res.results[c]["o_sq"], dtype=np.float64).sum()
    return np.asarray(total * scale, dtype=np.float32)

